# revision 1
# baseline (speedup 1.0000x reference)
"""Trainium2 Bass kernel for nn_LinearTemporalSelfAttention (B=4,T=8192,D=512,H=8).

Sharding: 8 cores = B(4) x T-halves(2). Each core owns a (b, t-half) slab
(4096 x 512) end-to-end. Cross-core data is only the KV-state einsum
(sum over full T) and the emb projection (emb_W sharded over TE within a
pair) — both folded into ONE pair-wise AllReduce of a 134 KB buffer.

Math notes (exact up to fp assoc):
 - softmax shift-invariance: exp(k)/sum(exp(k)) without max-subtraction
   (values are O(1) after LN with 0.02-scale weights).
 - k-mask (+ -1e6) is replaced by masking v (v*mask) and computing the
   softmax-T denominator S = sum_t exp(k)*mask via an extra column of
   ones*mask appended to v in the same PE matmul.
 - gamma/beta of LN1 are folded into Wq/Wk/Wv (+ biases) on the host.
 - attn normalization (1/S) is applied to the tiny (64,8,65) state, and
   the q-softmax denominator (1/sum) is applied to y after the q@attn
   matmul — so the big T-sized tensors never need normalizing passes.
"""
import numpy as np
import ml_dtypes

B, T, D, H, TE = 4, 8192, 512, 8, 2048
Dh = D // H          # 64
EPS = 1e-5
NCORES = 8
TH = T // 2          # 4096 rows per core
P = 128
NT = TH // P         # 32 row tiles
KC = D // P          # 4 contraction chunks
TEH = TE // 2        # 1024 te rows per core
TEC = TEH // P       # 8 te chunks
CCU = 64 * H * (Dh + 1)     # 33280 floats of U_aug
CCN = CCU + 2 * D           # + emb partial

_CACHE: dict = {}


def _build(flags):
    has_bq, has_bk, has_bv, has_outb, has_embb = flags
    from contextlib import ExitStack
    import concourse.bass as bass
    import concourse.bacc as bacc
    import concourse.tile as tile
    import concourse.mybir as mybir
    from concourse.masks import make_identity

    f32 = mybir.dt.float32
    bf16 = mybir.dt.bfloat16
    Alu = mybir.AluOpType
    Act = mybir.ActivationFunctionType

    nc = bacc.Bacc("TRN2", target_bir_lowering=False, debug=False,
                   enable_asserts=True, num_devices=NCORES)

    x_in = nc.declare_dram_parameter("x", [TH, D], f32, isOutput=False)
    mk_in = nc.declare_dram_parameter("mask", [TH], f32, isOutput=False)
    emb_in = nc.declare_dram_parameter("embv", [TEH], f32, isOutput=False)
    wq_in = nc.declare_dram_parameter("wq", [KC, P, D], bf16, isOutput=False)
    wk_in = nc.declare_dram_parameter("wk", [KC, P, D], bf16, isOutput=False)
    wv_in = nc.declare_dram_parameter("wv", [KC, P, D], bf16, isOutput=False)
    wo_in = nc.declare_dram_parameter("wo", [KC, P, D], bf16, isOutput=False)
    we_in = nc.declare_dram_parameter("we", [TEC, P, 2 * D], bf16, isOutput=False)
    vec_in = nc.declare_dram_parameter("vecs", [1, 8, D], f32, isOutput=False)
    y_out = nc.declare_dram_parameter("y", [TH, D], f32, isOutput=True)

    PAIRS = [[0, 1], [2, 3], [4, 5], [6, 7]]

    with tile.TileContext(nc) as tc, ExitStack() as ctx:
        const = ctx.enter_context(tc.tile_pool(name="const", bufs=1))
        wpool = ctx.enter_context(tc.tile_pool(name="wpool", bufs=1))
        xstash = ctx.enter_context(tc.tile_pool(name="xstash", bufs=NT))
        qstash = ctx.enter_context(tc.tile_pool(name="qstash", bufs=NT))
        dramp = ctx.enter_context(tc.tile_pool(name="dram", bufs=1, space="DRAM"))

        ident = const.tile([P, P], bf16)
        make_identity(nc, ident)
        eps_t = const.tile([P, 1], f32)
        nc.vector.memset(eps_t, EPS)
        ones8 = const.tile([P, H, 1], bf16)
        nc.vector.memset(ones8, 1.0)
        ones_row = const.tile([1, P], bf16)
        nc.vector.memset(ones_row, 1.0)

        wq_s = wpool.tile([P, KC, D], bf16)
        nc.sync.dma_start(out=wq_s, in_=wq_in[:].rearrange("c p d -> p c d"))
        wk_s = wpool.tile([P, KC, D], bf16)
        nc.sync.dma_start(out=wk_s, in_=wk_in[:].rearrange("c p d -> p c d"))
        wv_s = wpool.tile([P, KC, D], bf16)
        nc.sync.dma_start(out=wv_s, in_=wv_in[:].rearrange("c p d -> p c d"))
        wo_s = wpool.tile([P, KC, D], bf16)
        nc.sync.dma_start(out=wo_s, in_=wo_in[:].rearrange("c p d -> p c d"))
        we_s = wpool.tile([P, TEC, 2 * D], bf16)
        nc.sync.dma_start(out=we_s, in_=we_in[:].rearrange("c p d -> p c d"))
        mask_s = wpool.tile([P, NT], f32)
        nc.sync.dma_start(out=mask_s, in_=mk_in[:].rearrange("(n p) -> p n", p=P))
        vec_s = wpool.tile([1, 8, D], f32)
        nc.sync.dma_start(out=vec_s, in_=vec_in[:])

        cc_in_t = dramp.tile([CCN], f32)
        cc_out_t = dramp.tile([CCN], f32)

        x_tiles = []
        q_tiles = []

        with ExitStack() as ctxA:
            work = ctxA.enter_context(tc.tile_pool(name="work", bufs=3))
            psA = ctxA.enter_context(tc.tile_pool(name="psA", bufs=2, space="PSUM"))
            psT = ctxA.enter_context(tc.tile_pool(name="psT", bufs=2, space="PSUM"))
            psU = ctxA.enter_context(tc.tile_pool(name="psU", bufs=1, space="PSUM"))
            embp = ctxA.enter_context(tc.tile_pool(name="embp", bufs=1))

            # ---- bias broadcast tiles (only when biases nonzero) ----
            # broadcast row -> [P, D] via PE: ones[1,P].T @ row[1,D]
            def bcast_row(row_idx, name):
                pb = psT.tile([P, D], f32, tag="pT")
                rbf = const.tile([1, D], bf16, tag="rbf_" + name)
                nc.vector.tensor_copy(out=rbf, in_=vec_s[:, row_idx, :])
                nc.tensor.matmul(out=pb, lhsT=ones_row, rhs=rbf,
                                 start=True, stop=True)
                bc = const.tile([P, D], f32, tag="bc_" + name)
                nc.scalar.copy(out=bc, in_=pb)
                return bc

            bq_bc = bcast_row(0, "bq") if has_bq else None
            bk_bc = bcast_row(1, "bk") if has_bk else None
            bv_bc = bcast_row(2, "bv") if has_bv else None
            ob_bc = bcast_row(3, "ob") if has_outb else None

            # ---- emb projection partial (this core's TE shard) ----
            embt = embp.tile([P, TEC], f32)
            nc.sync.dma_start(out=embt, in_=emb_in[:].rearrange("(c p) -> p c", p=P))
            embsg = embp.tile([P, TEC], f32)
            nc.scalar.activation(out=embsg, in_=embt, func=Act.Sigmoid)
            embs = embp.tile([P, TEC], bf16)
            nc.vector.tensor_mul(out=embs, in0=embt, in1=embsg)
            pe0 = psA.tile([1, D], f32, tag="pq")
            pe1 = psA.tile([1, D], f32, tag="pk")
            for j in range(TEC):
                nc.tensor.matmul(out=pe0, lhsT=embs[:, j:j + 1],
                                 rhs=we_s[:, j, 0:D],
                                 start=(j == 0), stop=(j == TEC - 1))
            for j in range(TEC):
                nc.tensor.matmul(out=pe1, lhsT=embs[:, j:j + 1],
                                 rhs=we_s[:, j, D:2 * D],
                                 start=(j == 0), stop=(j == TEC - 1))
            emb_part = embp.tile([1, 2 * D], f32)
            nc.scalar.copy(out=emb_part[:, 0:D], in_=pe0)
            nc.scalar.copy(out=emb_part[:, D:2 * D], in_=pe1)

            u0 = psU.tile([64, 4, Dh + 1], f32, tag="u0")
            u1 = psU.tile([64, 4, Dh + 1], f32, tag="u1")

            # ---- phase A: LN, QKV projections, exp, U accumulation ----
            # ACT uses ONLY the ln/exp table (rstd = exp(-0.5*ln(var+eps)))
            # so no ACT_TABLE_LOAD ever fires after the first one.
            for i in range(NT):
                xt = xstash.tile([P, D], f32, tag="x")
                x_tiles.append(xt)
                nc.sync.dma_start(out=xt, in_=x_in[i * P:(i + 1) * P, :])
                st = work.tile([P, 6], f32, tag="st")
                nc.vector.bn_stats(out=st, in_=xt)
                mv = work.tile([P, 2], f32, tag="mv")
                nc.vector.bn_aggr(out=mv, in_=st)
                sd = work.tile([P, 1], f32, tag="sd")
                nc.scalar.activation(out=sd, in_=mv[:, 1:2], func=Act.Ln,
                                     bias=eps_t)
                rstd = work.tile([P, 1], f32, tag="rstd")
                nc.scalar.activation(out=rstd, in_=sd, func=Act.Exp,
                                     scale=-0.5)
                xn = work.tile([P, D], bf16, tag="xn")
                nc.vector.tensor_scalar(out=xn, in0=xt, scalar1=mv[:, 0:1],
                                        scalar2=rstd, op0=Alu.subtract,
                                        op1=Alu.mult)
                xT = work.tile([P, KC, P], bf16, tag="xT")
                for j in range(KC):
                    nc.sync.dma_start(out=xT[:, j, :],
                                      in_=xn[:, j * P:(j + 1) * P],
                                      transpose=True)

                pq = psA.tile([P, D], f32, tag="pq")
                pk = psA.tile([P, D], f32, tag="pk")
                pv = psA.tile([P, D], f32, tag="pv")
                for j in range(KC):
                    nc.tensor.matmul(out=pq, lhsT=xT[:, j, :], rhs=wq_s[:, j, :],
                                     start=(j == 0), stop=(j == KC - 1))
                    nc.tensor.matmul(out=pk, lhsT=xT[:, j, :], rhs=wk_s[:, j, :],
                                     start=(j == 0), stop=(j == KC - 1))
                    nc.tensor.matmul(out=pv, lhsT=xT[:, j, :], rhs=wv_s[:, j, :],
                                     start=(j == 0), stop=(j == KC - 1))
                if has_bq:
                    nc.vector.tensor_add(out=pq, in0=pq, in1=bq_bc)
                if has_bk:
                    nc.vector.tensor_add(out=pk, in0=pk, in1=bk_bc)
                if has_bv:
                    nc.vector.tensor_add(out=pv, in0=pv, in1=bv_bc)

                qt = qstash.tile([P, D], bf16, tag="qt")
                q_tiles.append(qt)
                nc.scalar.activation(out=qt, in_=pq, func=Act.Exp)

                et = work.tile([P, D], bf16, tag="et")
                nc.scalar.activation(out=et, in_=pk, func=Act.Exp)

                va = work.tile([P, H, Dh + 1], bf16, tag="va")
                nc.vector.tensor_scalar_mul(
                    out=va[:, :, 0:Dh],
                    in0=pv[:].rearrange("p (h d) -> p h d", h=H),
                    scalar1=mask_s[:, i:i + 1])
                nc.vector.tensor_scalar_mul(out=va[:, :, Dh:Dh + 1], in0=ones8,
                                            scalar1=mask_s[:, i:i + 1])
                for h in range(H):
                    u = u0 if h < 4 else u1
                    # one accumulation group per PSUM bank: start clears the
                    # whole zero-region once; has_written bits make the first
                    # write to each head slot an overwrite, later ones adds.
                    nc.tensor.matmul(out=u[:, h % 4, :],
                                     lhsT=et[:, h * Dh:(h + 1) * Dh],
                                     rhs=va[:, h, :],
                                     start=(i == 0 and h % 4 == 0),
                                     stop=(i == NT - 1 and h % 4 == 3))

            # ---- ship partials through the pair AllReduce ----
            u_sb = embp.tile([64, H, Dh + 1], f32)
            nc.scalar.copy(out=u_sb[:, 0:4, :], in_=u0)
            nc.scalar.copy(out=u_sb[:, 4:8, :], in_=u1)
            nc.sync.dma_start(
                out=cc_in_t[0:CCU].rearrange("(p h f) -> p h f", p=64, h=H),
                in_=u_sb)
            nc.sync.dma_start(
                out=cc_in_t[CCU:CCN].rearrange("(a f) -> a f", a=1),
                in_=emb_part)
            nc.gpsimd.collective_compute(
                "AllReduce", Alu.add, replica_groups=PAIRS,
                ins=[cc_in_t[:]], outs=[cc_out_t[:]])

        # ---- phase B prologue: attn state + stylization vectors ----
        with ExitStack() as ctxB:
            workB = ctxB.enter_context(tc.tile_pool(name="workB", bufs=3))
            psB = ctxB.enter_context(tc.tile_pool(name="psB", bufs=2, space="PSUM"))
            embB = ctxB.enter_context(tc.tile_pool(name="embB", bufs=1))

            # U state duplicated on both partition halves; attn2 is the
            # block-diagonal per-pair layout for the merged y matmuls:
            # attn2[:, p, :] = [[attn_{2p}, 0], [0, attn_{2p+1}]]
            u_f = embB.tile([P, H, Dh + 1], f32)
            nc.sync.dma_start(
                out=u_f[0:64], in_=cc_out_t[0:CCU].rearrange(
                    "(p h f) -> p h f", p=64, h=H))
            nc.sync.dma_start(
                out=u_f[64:P], in_=cc_out_t[0:CCU].rearrange(
                    "(p h f) -> p h f", p=64, h=H))
            emb_f = embB.tile([1, 2 * D], f32)
            nc.sync.dma_start(
                out=emb_f, in_=cc_out_t[CCU:CCN].rearrange("(a f) -> a f", a=1))

            rs = embB.tile([P, H, 1], f32)
            nc.vector.reciprocal(out=rs, in_=u_f[:, :, Dh:Dh + 1])
            attn2 = embB.tile([P, KC, P], bf16)
            nc.gpsimd.memset(attn2, 0.0)
            for h in range(H):
                base = 64 * (h % 2)
                nc.vector.tensor_scalar_mul(
                    out=attn2[base:base + 64, h // 2, base:base + 64],
                    in0=u_f[base:base + 64, h, 0:Dh],
                    scalar1=rs[base:base + 64, h, :])

            srow = embB.tile([1, D], f32)
            shrow = embB.tile([1, D], f32)
            if has_embb:
                nc.vector.tensor_add(out=srow, in0=emb_f[:, 0:D],
                                     in1=vec_s[:, 6, :])
                nc.vector.tensor_add(out=shrow, in0=emb_f[:, D:2 * D],
                                     in1=vec_s[:, 7, :])
            else:
                nc.vector.tensor_copy(out=srow, in_=emb_f[:, 0:D])
                nc.vector.tensor_copy(out=shrow, in_=emb_f[:, D:2 * D])
            t1 = embB.tile([1, D], f32)
            nc.vector.tensor_scalar_add(out=t1, in0=srow, scalar1=1.0)
            arow = embB.tile([1, D], bf16)
            nc.vector.tensor_mul(out=arow, in0=t1, in1=vec_s[:, 4, :])
            crow_f = embB.tile([1, D], f32)
            nc.vector.tensor_mul(out=crow_f, in0=t1, in1=vec_s[:, 5, :])
            nc.vector.tensor_add(out=crow_f, in0=crow_f, in1=shrow)
            crow = embB.tile([1, D], bf16)
            nc.vector.tensor_copy(out=crow, in_=crow_f)

            # broadcast a,c rows to [P, D] via PE ones-outer-product
            pa = psB.tile([P, D], f32, tag="py")
            nc.tensor.matmul(out=pa, lhsT=ones_row, rhs=arow,
                             start=True, stop=True)
            a_bc = embB.tile([P, D], f32)
            nc.scalar.copy(out=a_bc, in_=pa)
            pc = psB.tile([P, D], f32, tag="py")
            nc.tensor.matmul(out=pc, lhsT=ones_row, rhs=crow,
                             start=True, stop=True)
            c_bc = embB.tile([P, D], f32)
            nc.scalar.copy(out=c_bc, in_=pc)

            # ---- phase B: y = q@attn, LN2, stylize, silu, out proj ----
            for i in range(NT):
                qt = q_tiles[i]
                qTt = workB.tile([P, KC, P], bf16, tag="qTt")
                for j in range(KC):
                    nc.sync.dma_start(out=qTt[:, j, :],
                                      in_=qt[:, j * P:(j + 1) * P],
                                      transpose=True)
                py = psB.tile([P, KC, P], f32, tag="py")
                for j in range(KC):
                    nc.tensor.matmul(out=py[:, j, :], lhsT=qTt[:, j, :],
                                     rhs=attn2[:, j, :], start=True, stop=True)
                # q-softmax denominator + evacuate py with ACT copy*scale
                qs = workB.tile([P, H, 1], f32, tag="qs")
                nc.vector.reduce_sum(
                    out=qs, in_=qt[:].rearrange("p (h d) -> p h d", h=H),
                    axis=mybir.AxisListType.X)
                rq = workB.tile([P, H], f32, tag="rq")
                nc.vector.reciprocal(out=rq, in_=qs[:, :, 0])
                py_flat = py[:].rearrange("p a b -> p (a b)")
                ysb = workB.tile([P, D], f32, tag="ysb")
                for h in range(H):
                    nc.scalar.activation(out=ysb[:, h * Dh:(h + 1) * Dh],
                                         in_=py_flat[:, h * Dh:(h + 1) * Dh],
                                         func=Act.Copy,
                                         scale=rq[:, h:h + 1])
                st2 = workB.tile([P, 6], f32, tag="st2")
                nc.vector.bn_stats(out=st2, in_=ysb)
                mv2 = workB.tile([P, 2], f32, tag="mv2")
                nc.vector.bn_aggr(out=mv2, in_=st2)
                sd2 = workB.tile([P, 1], f32, tag="sd2")
                nc.scalar.activation(out=sd2, in_=mv2[:, 1:2], func=Act.Ln,
                                     bias=eps_t)
                rstd2 = workB.tile([P, 1], f32, tag="rstd2")
                nc.scalar.activation(out=rstd2, in_=sd2, func=Act.Exp,
                                     scale=-0.5)
                # in-place: ysb -> z2 -> h1 (saves SBUF)
                nc.vector.tensor_scalar(out=ysb, in0=ysb, scalar1=mv2[:, 0:1],
                                        scalar2=rstd2, op0=Alu.subtract,
                                        op1=Alu.mult)
                nc.gpsimd.tensor_mul(out=ysb, in0=ysb, in1=a_bc)
                nc.gpsimd.tensor_add(out=ysb, in0=ysb, in1=c_bc)
                # silu(x) = x / (1 + exp(-x)) — keeps ACT on the exp table
                eneg = workB.tile([P, D], f32, tag="eneg")
                nc.scalar.activation(out=eneg, in_=ysb, func=Act.Exp,
                                     scale=-1.0)
                nc.gpsimd.tensor_scalar_add(out=eneg, in0=eneg, scalar1=1.0)
                nc.vector.reciprocal(out=eneg, in_=eneg)
                hs = workB.tile([P, D], bf16, tag="hs")
                nc.gpsimd.tensor_mul(out=hs, in0=ysb, in1=eneg)
                hT = workB.tile([P, KC, P], bf16, tag="hT")
                for j in range(KC):
                    nc.sync.dma_start(out=hT[:, j, :],
                                      in_=hs[:, j * P:(j + 1) * P],
                                      transpose=True)
                po = psB.tile([P, D], f32, tag="po")
                for j in range(KC):
                    nc.tensor.matmul(out=po, lhsT=hT[:, j, :],
                                     rhs=wo_s[:, j, :],
                                     start=(j == 0), stop=(j == KC - 1))
                osb = workB.tile([P, D], f32, tag="osb")
                nc.vector.tensor_add(out=osb, in0=po, in1=x_tiles[i])
                if has_outb:
                    nc.vector.tensor_add(out=osb, in0=osb, in1=ob_bc)
                nc.sync.dma_start(out=y_out[i * P:(i + 1) * P, :], in_=osb)

    nc.compile()
    return nc


def _prep(inputs, flags):
    bf = ml_dtypes.bfloat16
    x = np.asarray(inputs["x"], np.float32)
    emb = np.asarray(inputs["emb"], np.float32)
    src_mask = np.asarray(inputs["src_mask"], np.float32)
    gamma = np.asarray(inputs["gamma"], np.float32)
    beta = np.asarray(inputs["beta"], np.float32)
    gamma2 = np.asarray(inputs["gamma2"], np.float32)
    beta2 = np.asarray(inputs["beta2"], np.float32)
    emb_b = np.asarray(inputs["emb_b"], np.float32)
    out_b = np.asarray(inputs["out_b"], np.float32)

    def foldW(Wname):
        W = np.asarray(inputs[Wname], np.float32)
        return np.ascontiguousarray(
            (gamma[:, None] * W).astype(bf).reshape(KC, P, D))

    wq, wk, wv = foldW("Wq"), foldW("Wk"), foldW("Wv")
    wo = np.ascontiguousarray(
        np.asarray(inputs["out_W"], np.float32).astype(bf).reshape(KC, P, D))
    bq_f = np.asarray(inputs["bq"], np.float32) + beta @ np.asarray(inputs["Wq"], np.float32)
    bk_f = np.asarray(inputs["bk"], np.float32) + beta @ np.asarray(inputs["Wk"], np.float32)
    bv_f = np.asarray(inputs["bv"], np.float32) + beta @ np.asarray(inputs["Wv"], np.float32)
    vecs = np.ascontiguousarray(np.stack(
        [bq_f, bk_f, bv_f, out_b, gamma2, beta2, emb_b[:D], emb_b[D:]]
    ).astype(np.float32).reshape(1, 8, D))
    emb_W = np.asarray(inputs["emb_W"], np.float32)
    we_halves = [
        np.ascontiguousarray(
            emb_W[t * TEH:(t + 1) * TEH].astype(bf).reshape(TEC, P, 2 * D))
        for t in range(2)]

    in_maps = []
    for c in range(NCORES):
        b, th = c // 2, c % 2
        sl = slice(th * TH, (th + 1) * TH)
        in_maps.append({
            "x": np.ascontiguousarray(x[b, sl]),
            "mask": np.ascontiguousarray(src_mask[b, sl, 0]),
            "embv": np.ascontiguousarray(emb[b, th * TEH:(th + 1) * TEH]),
            "wq": wq, "wk": wk, "wv": wv, "wo": wo,
            "we": we_halves[th],
            "vecs": vecs,
        })
    return in_maps


def _flags(inputs):
    gamma = np.asarray(inputs["gamma"], np.float32)
    beta = np.asarray(inputs["beta"], np.float32)

    def nz(v):
        return bool(np.any(np.asarray(v) != 0))

    bq_f = np.asarray(inputs["bq"], np.float32) + beta @ np.asarray(inputs["Wq"], np.float32)
    bk_f = np.asarray(inputs["bk"], np.float32) + beta @ np.asarray(inputs["Wk"], np.float32)
    bv_f = np.asarray(inputs["bv"], np.float32) + beta @ np.asarray(inputs["Wv"], np.float32)
    return (nz(bq_f), nz(bk_f), nz(bv_f), nz(inputs["out_b"]), nz(inputs["emb_b"]))


def get_nc_and_inmaps(**inputs):
    flags = _flags(inputs)
    if flags not in _CACHE:
        _CACHE[flags] = _build(flags)
    return _CACHE[flags], _prep(inputs, flags)


def kernel(**inputs):
    from concourse.bass_utils import run_bass_kernel_spmd
    nc, in_maps = get_nc_and_inmaps(**inputs)
    res = run_bass_kernel_spmd(nc, in_maps, list(range(NCORES)))
    out = np.empty((B, T, D), np.float32)
    for c in range(NCORES):
        b, th = c // 2, c % 2
        out[b, th * TH:(th + 1) * TH] = res.results[c]["y"]
    return out



# revision 2
# speedup vs baseline: 2.1529x; 2.1529x over previous
"""Trainium2 Bass kernel for nn_LinearTemporalSelfAttention (B=4,T=8192,D=512,H=8).

Sharding: 8 cores = B(4) x T-halves(2). Each core owns a (b, t-half) slab
(4096 x 512) end-to-end. Cross-core data is only the KV-state einsum
(sum over full T) and the emb projection (emb_W sharded over TE within a
pair) — both folded into ONE pair-wise AllReduce of a 134 KB buffer.

v2 performance structure (vs v1 baseline):
 - LN rstd is computed BATCHED for all 32 row-tiles in one Ln + one Exp
   (the per-tile Ln/Exp alternation forced an ACT_TABLE_LOAD (1.28us)
   twice per tile in v1 — 160us total).
 - All transposes are PE transposes (matmul is_transpose) + PSUM evac
   instead of DMA_TRANSPOSE (1.2us each, 384 of them in v1).
 - QKV and out projections run in fp8(e4m3) DoubleRow perf mode
   (2 k-subtiles per pass = 2x matmul throughput). Weights are scaled
   x64 on host so 0.02-scale entries sit in e4m3's normal range; the
   1/64 descale folds into existing ACT scales / tensor_scalar ops.
 - Phase B is two passes: B1 (y matmul + 1/S evac + bn_stats) and B2
   (normalize+stylize+silu+out-proj), with LN2 rstd batched between
   them; silu uses the native ACT Silu (one table load total).
 - Elementwise work is spread across scalar/vector/gpsimd so no engine
   exceeds ~60% (v1 phase B was gpsimd-bound at 77%).
"""
import numpy as np
import ml_dtypes

B, T, D, H, TE = 4, 8192, 512, 8, 2048
Dh = D // H          # 64
EPS = 1e-5
NCORES = 8
TH = T // 2          # 4096 rows per core
P = 128
NT = TH // P         # 32 row tiles
KC = D // P          # 4 contraction chunks
TEH = TE // 2        # 1024 te rows per core
TEC = TEH // P       # 8 te chunks
CCU = 64 * H * (Dh + 1)     # 33280 floats of U_aug
CCN = CCU + 2 * D           # + emb partial
WSCALE = 64.0        # fp8 weight prescale
RWS = 1.0 / WSCALE

_CACHE: dict = {}


def _build(flags):
    has_bq, has_bk, has_bv, has_outb, has_embb = flags
    from contextlib import ExitStack
    import concourse.bass as bass
    import concourse.bacc as bacc
    import concourse.tile as tile
    import concourse.mybir as mybir
    from concourse.masks import make_identity

    f32 = mybir.dt.float32
    bf16 = mybir.dt.bfloat16
    f8 = mybir.dt.float8e4
    Alu = mybir.AluOpType
    Act = mybir.ActivationFunctionType
    DR = mybir.MatmulPerfMode.DoubleRow

    nc = bacc.Bacc("TRN2", target_bir_lowering=False, debug=False,
                   enable_asserts=True, num_devices=NCORES)

    x_in = nc.declare_dram_parameter("x", [TH, D], f32, isOutput=False)
    mk_in = nc.declare_dram_parameter("mask", [TH], f32, isOutput=False)
    emb_in = nc.declare_dram_parameter("embv", [TEH], f32, isOutput=False)
    wq_in = nc.declare_dram_parameter("wq", [KC, P, D], f8, isOutput=False)
    wk_in = nc.declare_dram_parameter("wk", [KC, P, D], f8, isOutput=False)
    wv_in = nc.declare_dram_parameter("wv", [KC, P, D], f8, isOutput=False)
    wo_in = nc.declare_dram_parameter("wo", [KC, P, D], f8, isOutput=False)
    we_in = nc.declare_dram_parameter("we", [TEC, P, 2 * D], bf16, isOutput=False)
    vec_in = nc.declare_dram_parameter("vecs", [1, 8, D], f32, isOutput=False)
    y_out = nc.declare_dram_parameter("y", [TH, D], f32, isOutput=True)

    PAIRS = [[0, 1], [2, 3], [4, 5], [6, 7]]

    with tile.TileContext(nc) as tc, ExitStack() as ctx:
        const = ctx.enter_context(tc.tile_pool(name="const", bufs=1))
        wpool = ctx.enter_context(tc.tile_pool(name="wpool", bufs=1))
        xstash = ctx.enter_context(tc.tile_pool(name="xstash", bufs=NT))
        qstash = ctx.enter_context(tc.tile_pool(name="qstash", bufs=NT))
        ystash = ctx.enter_context(tc.tile_pool(name="ystash", bufs=NT))
        stat = ctx.enter_context(tc.tile_pool(name="stat", bufs=1))
        dramp = ctx.enter_context(tc.tile_pool(name="dram", bufs=1, space="DRAM"))

        ident = const.tile([P, P], bf16)
        make_identity(nc, ident)
        eps_t = const.tile([P, 1], f32)
        nc.vector.memset(eps_t, EPS)
        ones8 = const.tile([P, H, 1], bf16)
        nc.vector.memset(ones8, 1.0)
        ones_row = const.tile([1, P], bf16)
        nc.vector.memset(ones_row, 1.0)

        wq_s = wpool.tile([P, KC, D], f8)
        nc.sync.dma_start(out=wq_s, in_=wq_in[:].rearrange("c p d -> p c d"))
        wk_s = wpool.tile([P, KC, D], f8)
        nc.sync.dma_start(out=wk_s, in_=wk_in[:].rearrange("c p d -> p c d"))
        wv_s = wpool.tile([P, KC, D], f8)
        nc.sync.dma_start(out=wv_s, in_=wv_in[:].rearrange("c p d -> p c d"))
        wo_s = wpool.tile([P, KC, D], f8)
        nc.sync.dma_start(out=wo_s, in_=wo_in[:].rearrange("c p d -> p c d"))
        we_s = wpool.tile([P, TEC, 2 * D], bf16)
        nc.sync.dma_start(out=we_s, in_=we_in[:].rearrange("c p d -> p c d"))
        mask_s = wpool.tile([P, NT], f32)
        nc.sync.dma_start(out=mask_s, in_=mk_in[:].rearrange("(n p) -> p n", p=P))
        vec_s = wpool.tile([1, 8, D], f32)
        nc.sync.dma_start(out=vec_s, in_=vec_in[:])

        # persistent stat tiles (subtile deps let slices pipeline)
        mv_st = stat.tile([P, NT, 2], f32)
        mv2_st = stat.tile([P, NT, 2], f32)
        rq_st = stat.tile([P, NT, H], f32)
        rstd_a = stat.tile([P, NT], f32)
        nb_a = stat.tile([P, NT], f32)
        rstd2_a = stat.tile([P, NT], f32)
        nb2_a = stat.tile([P, NT], f32)

        cc_in_t = dramp.tile([CCN], f32)
        cc_out_t = dramp.tile([CCN], f32)

        x_tiles = []
        q_tiles = []
        y_tiles = []

        with ExitStack() as ctxA:
            work = ctxA.enter_context(tc.tile_pool(name="work", bufs=3))
            psP = ctxA.enter_context(tc.tile_pool(name="psP", bufs=1, space="PSUM"))
            psT = ctxA.enter_context(tc.tile_pool(name="psT", bufs=2, space="PSUM"))
            psU = ctxA.enter_context(tc.tile_pool(name="psU", bufs=1, space="PSUM"))
            embp = ctxA.enter_context(tc.tile_pool(name="embp", bufs=1))

            # ---- bias broadcast tiles (only when biases nonzero) ----
            def bcast_row(row_idx, name):
                pb = psT.tile([P, KC, P], f32, tag="pT")
                rbf = const.tile([1, D], bf16, tag="rbf_" + name)
                nc.vector.tensor_copy(out=rbf, in_=vec_s[:, row_idx, :])
                nc.tensor.matmul(out=pb[:].rearrange("p a b -> p (a b)"),
                                 lhsT=ones_row, rhs=rbf, start=True, stop=True)
                bc = const.tile([P, D], f32, tag="bc_" + name)
                nc.scalar.copy(out=bc, in_=pb[:].rearrange("p a b -> p (a b)"))
                return bc

            bq_bc = bcast_row(0, "bq") if has_bq else None
            bk_bc = bcast_row(1, "bk") if has_bk else None
            bv_bc = bcast_row(2, "bv") if has_bv else None
            ob_bc = bcast_row(3, "ob") if has_outb else None

            # ---- emb projection partial (this core's TE shard) ----
            # silu via exp-table ops only: e*sigmoid(e) = e/(1+exp(-e))
            embt = embp.tile([P, TEC], f32)
            nc.sync.dma_start(out=embt, in_=emb_in[:].rearrange("(c p) -> p c", p=P))
            emneg = embp.tile([P, TEC], f32)
            nc.scalar.activation(out=emneg, in_=embt, func=Act.Exp, scale=-1.0)
            nc.vector.tensor_scalar_add(out=emneg, in0=emneg, scalar1=1.0)
            nc.vector.reciprocal(out=emneg, in_=emneg)
            embs = embp.tile([P, TEC], bf16)
            nc.vector.tensor_mul(out=embs, in0=embt, in1=emneg)
            pe0 = psP.tile([P, D], f32, tag="pq")
            pe1 = psP.tile([P, D], f32, tag="pk")
            for j in range(TEC):
                nc.tensor.matmul(out=pe0[0:1, :], lhsT=embs[:, j:j + 1],
                                 rhs=we_s[:, j, 0:D],
                                 start=(j == 0), stop=(j == TEC - 1))
            for j in range(TEC):
                nc.tensor.matmul(out=pe1[0:1, :], lhsT=embs[:, j:j + 1],
                                 rhs=we_s[:, j, D:2 * D],
                                 start=(j == 0), stop=(j == TEC - 1))
            emb_part = embp.tile([1, 2 * D], f32)
            nc.scalar.copy(out=emb_part[:, 0:D], in_=pe0[0:1, :])
            nc.scalar.copy(out=emb_part[:, D:2 * D], in_=pe1[0:1, :])

            # ---- A0: load x, batched LN stats ----
            for i in range(NT):
                xt = xstash.tile([P, D], f32, tag="x")
                x_tiles.append(xt)
                nc.sync.dma_start(out=xt, in_=x_in[i * P:(i + 1) * P, :])
                st = work.tile([P, 6], f32, tag="st")
                nc.vector.bn_stats(out=st, in_=xt)
                nc.vector.bn_aggr(out=mv_st[:, i, :], in_=st)

            # batched rstd/bias: rstd = exp(-0.5*ln(var+eps)); nb = -mu*rstd
            sd_a = stat.tile([P, NT], f32)
            nc.scalar.activation(out=sd_a, in_=mv_st[:, :, 1], func=Act.Ln,
                                 bias=eps_t)
            nc.scalar.activation(out=rstd_a, in_=sd_a, func=Act.Exp, scale=-0.5)
            nc.vector.tensor_mul(out=nb_a, in0=mv_st[:, :, 0], in1=rstd_a)
            nc.vector.tensor_scalar_mul(out=nb_a, in0=nb_a, scalar1=-1.0)

            u0 = psU.tile([P, 2, Dh + 1 + Dh + 1], f32, tag="u0")
            u1 = psU.tile([P, 2, Dh + 1 + Dh + 1], f32, tag="u1")

            # ---- A1: normalize, QKV (fp8 DoubleRow), exp, U accumulation ----
            for i in range(NT):
                xn = work.tile([P, D], bf16, tag="xn")
                nc.scalar.activation(out=xn, in_=x_tiles[i], func=Act.Identity,
                                     scale=rstd_a[:, i:i + 1],
                                     bias=nb_a[:, i:i + 1])
                xTp = psT.tile([P, KC, P], bf16, tag="pT")
                for j in range(KC):
                    nc.tensor.transpose(xTp[:, j, :], xn[:, j * P:(j + 1) * P],
                                        ident)
                xT = work.tile([P, KC, P], f8, tag="xT")
                for j in range(KC):
                    if j % 2 == 0:
                        nc.scalar.copy(out=xT[:, j, :], in_=xTp[:, j, :])
                    else:
                        nc.vector.tensor_copy(out=xT[:, j, :], in_=xTp[:, j, :])

                pq = psP.tile([P, D], f32, tag="pq")
                pk = psP.tile([P, D], f32, tag="pk")
                pv = psP.tile([P, D], f32, tag="pv")
                for m in range(2):
                    nc.tensor.matmul(out=pq, lhsT=xT[:, 2 * m:2 * m + 2, :],
                                     rhs=wq_s[:, 2 * m:2 * m + 2, :],
                                     start=(m == 0), stop=(m == 1),
                                     perf_mode=DR)
                for m in range(2):
                    nc.tensor.matmul(out=pk, lhsT=xT[:, 2 * m:2 * m + 2, :],
                                     rhs=wk_s[:, 2 * m:2 * m + 2, :],
                                     start=(m == 0), stop=(m == 1),
                                     perf_mode=DR)
                for m in range(2):
                    nc.tensor.matmul(out=pv, lhsT=xT[:, 2 * m:2 * m + 2, :],
                                     rhs=wv_s[:, 2 * m:2 * m + 2, :],
                                     start=(m == 0), stop=(m == 1),
                                     perf_mode=DR)
                if has_bq:
                    nc.vector.scalar_tensor_tensor(
                        out=pq, in0=pq, scalar=RWS, in1=bq_bc,
                        op0=Alu.mult, op1=Alu.add)
                if has_bk:
                    nc.vector.scalar_tensor_tensor(
                        out=pk, in0=pk, scalar=RWS, in1=bk_bc,
                        op0=Alu.mult, op1=Alu.add)
                if has_bv:
                    nc.vector.scalar_tensor_tensor(
                        out=pv, in0=pv, scalar=RWS, in1=bv_bc,
                        op0=Alu.mult, op1=Alu.add)
                qsc = 1.0 if has_bq else RWS
                ksc = 1.0 if has_bk else RWS
                vsc = 1.0 if has_bv else RWS

                # q: exp, per-head sums, PE transpose into qT stash
                qt = work.tile([P, D], bf16, tag="qt")
                nc.scalar.activation(out=qt, in_=pq, func=Act.Exp, scale=qsc)
                qs = work.tile([P, H, 1], f32, tag="qs")
                nc.vector.reduce_sum(
                    out=qs, in_=qt[:].rearrange("p (h d) -> p h d", h=H),
                    axis=mybir.AxisListType.X)
                nc.vector.reciprocal(out=rq_st[:, i, :], in_=qs[:, :, 0])
                qTp = psT.tile([P, KC, P], bf16, tag="pT")
                for j in range(KC):
                    nc.tensor.transpose(qTp[:, j, :], qt[:, j * P:(j + 1) * P],
                                        ident)
                qT = qstash.tile([P, KC, P], bf16, tag="qT")
                q_tiles.append(qT)
                for j in range(KC):
                    if j % 2 == 0:
                        nc.scalar.copy(out=qT[:, j, :], in_=qTp[:, j, :])
                    else:
                        nc.vector.tensor_copy(out=qT[:, j, :], in_=qTp[:, j, :])

                # k: exp; v: mask+descale, ones column
                et = work.tile([P, D], bf16, tag="et")
                nc.scalar.activation(out=et, in_=pk, func=Act.Exp, scale=ksc)
                va = work.tile([P, H, Dh + 1], bf16, tag="va")
                nc.vector.tensor_scalar(
                    out=va[:, :, 0:Dh],
                    in0=pv[:].rearrange("p (h d) -> p h d", h=H),
                    scalar1=mask_s[:, i:i + 1], scalar2=vsc,
                    op0=Alu.mult, op1=Alu.mult)
                nc.vector.tensor_scalar_mul(out=va[:, :, Dh:Dh + 1], in0=ones8,
                                            scalar1=mask_s[:, i:i + 1])
                # U per head-pair: lhsT = et 128-cols, rhs = two va heads
                for p2 in range(4):
                    u = u0 if p2 < 2 else u1
                    nc.tensor.matmul(out=u[:, p2 % 2, :],
                                     lhsT=et[:, p2 * P:(p2 + 1) * P],
                                     rhs=va[:, 2 * p2:2 * p2 + 2, :],
                                     start=(i == 0 and p2 % 2 == 0),
                                     stop=(i == NT - 1 and p2 % 2 == 1))

            # ---- ship partials through the pair AllReduce ----
            # diag blocks of each head-pair tile -> u_sb [64, H, 65]
            u_sb = embp.tile([64, H, Dh + 1], f32)
            for p2 in range(4):
                u = u0 if p2 < 2 else u1
                nc.scalar.copy(out=u_sb[:, 2 * p2, :],
                               in_=u[0:64, p2 % 2, 0:Dh + 1])
                nc.scalar.copy(out=u_sb[:, 2 * p2 + 1, :],
                               in_=u[64:P, p2 % 2, Dh + 1:2 * Dh + 2])
            nc.sync.dma_start(
                out=cc_in_t[0:CCU].rearrange("(p h f) -> p h f", p=64, h=H),
                in_=u_sb)
            nc.sync.dma_start(
                out=cc_in_t[CCU:CCN].rearrange("(a f) -> a f", a=1),
                in_=emb_part)
            nc.gpsimd.collective_compute(
                "AllReduce", Alu.add, replica_groups=PAIRS,
                ins=[cc_in_t[:]], outs=[cc_out_t[:]])

        # ---- phase B prologue: attn state + stylization vectors ----
        with ExitStack() as ctxB:
            workB = ctxB.enter_context(tc.tile_pool(name="workB", bufs=3))
            psB = ctxB.enter_context(tc.tile_pool(name="psB", bufs=2, space="PSUM"))
            embB = ctxB.enter_context(tc.tile_pool(name="embB", bufs=1))

            u_f = embB.tile([P, H, Dh + 1], f32)
            nc.sync.dma_start(
                out=u_f[0:64], in_=cc_out_t[0:CCU].rearrange(
                    "(p h f) -> p h f", p=64, h=H))
            nc.sync.dma_start(
                out=u_f[64:P], in_=cc_out_t[0:CCU].rearrange(
                    "(p h f) -> p h f", p=64, h=H))
            emb_f = embB.tile([1, 2 * D], f32)
            nc.sync.dma_start(
                out=emb_f, in_=cc_out_t[CCU:CCN].rearrange("(a f) -> a f", a=1))

            rs = embB.tile([P, H, 1], f32)
            nc.vector.reciprocal(out=rs, in_=u_f[:, :, Dh:Dh + 1])
            attn2 = embB.tile([P, KC, P], bf16)
            nc.gpsimd.memset(attn2, 0.0)
            for h in range(H):
                base = 64 * (h % 2)
                nc.vector.tensor_scalar_mul(
                    out=attn2[base:base + 64, h // 2, base:base + 64],
                    in0=u_f[base:base + 64, h, 0:Dh],
                    scalar1=rs[base:base + 64, h, :])

            srow = embB.tile([1, D], f32)
            shrow = embB.tile([1, D], f32)
            if has_embb:
                nc.vector.tensor_add(out=srow, in0=emb_f[:, 0:D],
                                     in1=vec_s[:, 6, :])
                nc.vector.tensor_add(out=shrow, in0=emb_f[:, D:2 * D],
                                     in1=vec_s[:, 7, :])
            else:
                nc.vector.tensor_copy(out=srow, in_=emb_f[:, 0:D])
                nc.vector.tensor_copy(out=shrow, in_=emb_f[:, D:2 * D])
            t1 = embB.tile([1, D], f32)
            nc.vector.tensor_scalar_add(out=t1, in0=srow, scalar1=1.0)
            arow = embB.tile([1, D], bf16)
            nc.vector.tensor_mul(out=arow, in0=t1, in1=vec_s[:, 4, :])
            crow_f = embB.tile([1, D], f32)
            nc.vector.tensor_mul(out=crow_f, in0=t1, in1=vec_s[:, 5, :])
            nc.vector.tensor_add(out=crow_f, in0=crow_f, in1=shrow)
            crow = embB.tile([1, D], bf16)
            nc.vector.tensor_copy(out=crow, in_=crow_f)

            # broadcast a,c rows to [P, D] via PE ones-outer-product
            pa = psB.tile([P, D], f32, tag="po")
            nc.tensor.matmul(out=pa, lhsT=ones_row, rhs=arow,
                             start=True, stop=True)
            a_bc = embB.tile([P, D], bf16)
            nc.scalar.copy(out=a_bc, in_=pa)
            pc = psB.tile([P, D], f32, tag="po")
            nc.tensor.matmul(out=pc, lhsT=ones_row, rhs=crow,
                             start=True, stop=True)
            c_bc = embB.tile([P, D], bf16)
            nc.scalar.copy(out=c_bc, in_=pc)

            # ---- B1: y = qn@attn (per-head 1/S on evac), LN2 stats ----
            for i in range(NT):
                py = psB.tile([P, KC, P], f32, tag="py")
                for j in range(KC):
                    nc.tensor.matmul(out=py[:, j, :], lhsT=q_tiles[i][:, j, :],
                                     rhs=attn2[:, j, :], start=True, stop=True)
                py_flat = py[:].rearrange("p a b -> p (a b)")
                ysb = ystash.tile([P, D], bf16, tag="ysb")
                y_tiles.append(ysb)
                for h in range(H):
                    sl = slice(h * Dh, (h + 1) * Dh)
                    if h % 2 == 0:
                        nc.scalar.activation(out=ysb[:, sl], in_=py_flat[:, sl],
                                             func=Act.Copy,
                                             scale=rq_st[:, i, h:h + 1])
                    else:
                        nc.vector.tensor_scalar_mul(
                            out=ysb[:, sl], in0=py_flat[:, sl],
                            scalar1=rq_st[:, i, h:h + 1])
                st2 = workB.tile([P, 6], f32, tag="st2")
                nc.vector.bn_stats(out=st2, in_=ysb)
                nc.vector.bn_aggr(out=mv2_st[:, i, :], in_=st2)

            # batched LN2 rstd/bias
            sd2_a = stat.tile([P, NT], f32)
            nc.scalar.activation(out=sd2_a, in_=mv2_st[:, :, 1], func=Act.Ln,
                                 bias=eps_t)
            nc.scalar.activation(out=rstd2_a, in_=sd2_a, func=Act.Exp,
                                 scale=-0.5)
            nc.vector.tensor_mul(out=nb2_a, in0=mv2_st[:, :, 0], in1=rstd2_a)
            nc.vector.tensor_scalar_mul(out=nb2_a, in0=nb2_a, scalar1=-1.0)

            # ---- B2: normalize, stylize, silu, out proj, residual ----
            for i in range(NT):
                z = workB.tile([P, D], bf16, tag="z")
                nc.vector.tensor_scalar(out=z, in0=y_tiles[i],
                                        scalar1=rstd2_a[:, i:i + 1],
                                        scalar2=nb2_a[:, i:i + 1],
                                        op0=Alu.mult, op1=Alu.add)
                nc.gpsimd.tensor_mul(out=z, in0=z, in1=a_bc)
                nc.vector.tensor_add(out=z, in0=z, in1=c_bc)
                hs = workB.tile([P, D], bf16, tag="hs")
                nc.scalar.activation(out=hs, in_=z, func=Act.Silu)
                hTp = psB.tile([P, KC, P], bf16, tag="pT")
                for j in range(KC):
                    nc.tensor.transpose(hTp[:, j, :], hs[:, j * P:(j + 1) * P],
                                        ident)
                hT = workB.tile([P, KC, P], f8, tag="hT")
                for j in range(KC):
                    if j % 2 == 0:
                        nc.scalar.copy(out=hT[:, j, :], in_=hTp[:, j, :])
                    else:
                        nc.vector.tensor_copy(out=hT[:, j, :], in_=hTp[:, j, :])
                po = psB.tile([P, D], f32, tag="po")
                for m in range(2):
                    nc.tensor.matmul(out=po, lhsT=hT[:, 2 * m:2 * m + 2, :],
                                     rhs=wo_s[:, 2 * m:2 * m + 2, :],
                                     start=(m == 0), stop=(m == 1),
                                     perf_mode=DR)
                osb = workB.tile([P, D], f32, tag="osb")
                nc.vector.scalar_tensor_tensor(
                    out=osb, in0=po, scalar=RWS, in1=x_tiles[i],
                    op0=Alu.mult, op1=Alu.add)
                if has_outb:
                    nc.vector.tensor_add(out=osb, in0=osb, in1=ob_bc)
                nc.sync.dma_start(out=y_out[i * P:(i + 1) * P, :], in_=osb)

    nc.compile()
    return nc


def _to_f8(a):
    return np.clip(a * WSCALE, -240.0, 240.0).astype(ml_dtypes.float8_e4m3fn)


def _prep(inputs, flags):
    bf = ml_dtypes.bfloat16
    x = np.asarray(inputs["x"], np.float32)
    emb = np.asarray(inputs["emb"], np.float32)
    src_mask = np.asarray(inputs["src_mask"], np.float32)
    gamma = np.asarray(inputs["gamma"], np.float32)
    beta = np.asarray(inputs["beta"], np.float32)
    gamma2 = np.asarray(inputs["gamma2"], np.float32)
    beta2 = np.asarray(inputs["beta2"], np.float32)
    emb_b = np.asarray(inputs["emb_b"], np.float32)
    out_b = np.asarray(inputs["out_b"], np.float32)

    def foldW(Wname):
        W = np.asarray(inputs[Wname], np.float32)
        return np.ascontiguousarray(_to_f8(gamma[:, None] * W).reshape(KC, P, D))

    wq, wk, wv = foldW("Wq"), foldW("Wk"), foldW("Wv")
    wo = np.ascontiguousarray(
        _to_f8(np.asarray(inputs["out_W"], np.float32)).reshape(KC, P, D))
    bq_f = np.asarray(inputs["bq"], np.float32) + beta @ np.asarray(inputs["Wq"], np.float32)
    bk_f = np.asarray(inputs["bk"], np.float32) + beta @ np.asarray(inputs["Wk"], np.float32)
    bv_f = np.asarray(inputs["bv"], np.float32) + beta @ np.asarray(inputs["Wv"], np.float32)
    vecs = np.ascontiguousarray(np.stack(
        [bq_f, bk_f, bv_f, out_b, gamma2, beta2, emb_b[:D], emb_b[D:]]
    ).astype(np.float32).reshape(1, 8, D))
    emb_W = np.asarray(inputs["emb_W"], np.float32)
    we_halves = [
        np.ascontiguousarray(
            emb_W[t * TEH:(t + 1) * TEH].astype(bf).reshape(TEC, P, 2 * D))
        for t in range(2)]

    in_maps = []
    for c in range(NCORES):
        b, th = c // 2, c % 2
        sl = slice(th * TH, (th + 1) * TH)
        in_maps.append({
            "x": np.ascontiguousarray(x[b, sl]),
            "mask": np.ascontiguousarray(src_mask[b, sl, 0]),
            "embv": np.ascontiguousarray(emb[b, th * TEH:(th + 1) * TEH]),
            "wq": wq, "wk": wk, "wv": wv, "wo": wo,
            "we": we_halves[th],
            "vecs": vecs,
        })
    return in_maps


def _flags(inputs):
    beta = np.asarray(inputs["beta"], np.float32)

    def nz(v):
        return bool(np.any(np.asarray(v) != 0))

    bq_f = np.asarray(inputs["bq"], np.float32) + beta @ np.asarray(inputs["Wq"], np.float32)
    bk_f = np.asarray(inputs["bk"], np.float32) + beta @ np.asarray(inputs["Wk"], np.float32)
    bv_f = np.asarray(inputs["bv"], np.float32) + beta @ np.asarray(inputs["Wv"], np.float32)
    return (nz(bq_f), nz(bk_f), nz(bv_f), nz(inputs["out_b"]), nz(inputs["emb_b"]))


def get_nc_and_inmaps(**inputs):
    flags = _flags(inputs)
    if flags not in _CACHE:
        _CACHE[flags] = _build(flags)
    return _CACHE[flags], _prep(inputs, flags)


def kernel(**inputs):
    from concourse.bass_utils import run_bass_kernel_spmd
    nc, in_maps = get_nc_and_inmaps(**inputs)
    res = run_bass_kernel_spmd(nc, in_maps, list(range(NCORES)))
    out = np.empty((B, T, D), np.float32)
    for c in range(NCORES):
        b, th = c // 2, c % 2
        out[b, th * TH:(th + 1) * TH] = res.results[c]["y"]
    return out


# revision 15
# speedup vs baseline: 2.4564x; 1.1410x over previous
"""Trainium2 Bass kernel for nn_LinearTemporalSelfAttention (B=4,T=8192,D=512,H=8).

Sharding: 8 cores = B(4) x T-halves(2). Each core owns a (b, t-half) slab
(4096 x 512) end-to-end. Cross-core data is only the KV-state einsum
(sum over full T) and the emb projection (emb_W sharded over TE within a
pair) — folded into two pair-wise bf16 AllReduces (first half of U mid-
phase-A so it overlaps compute, second half at the end).

v3 structure:
 - LN rstd batched per 16-tile group (one Ln + one Exp each) so the
   scalar engine never thrashes activation tables; groups interleave
   with the projection loop so stats DMA/vector work overlaps PE work.
 - All transposes on the PE (matmul is_transpose) + PSUM evac.
 - QKV / out projections and the KV-state einsum run fp8(e4m3)
   DoubleRow (2 k-subtiles per pass). Weights x64 on host; exp(k),
   masked v quantized to fp8 (their errors average out over T in U).
 - Per-head softmax-q 1/S applied in ONE broadcast tensor_tensor op.
 - Phase B pass 2 works in transposed space: PE-transpose LN2-normalized
   y, then a single ACT Silu per chunk applies the stylization scale
   and shift (per-partition there) while evacuating PSUM into fp8.
 - Elementwise work spread across scalar/vector/gpsimd.
"""
import numpy as np
import ml_dtypes

B, T, D, H, TE = 4, 8192, 512, 8, 2048
Dh = D // H          # 64
EPS = 1e-5
NCORES = 8
TH = T // 2          # 4096 rows per core
P = 128
NT = TH // P         # 32 row tiles
NG = 16              # tiles per stat/U group
KC = D // P          # 4 contraction chunks
TEH = TE // 2        # 1024 te rows per core
TEC = TEH // P       # 8 te chunks
CCU = 64 * H * (Dh + 1)     # 33280 floats of U_aug
CCN = CCU + 2 * D           # + emb partial
WSCALE = 64.0        # fp8 weight prescale
RWS = 1.0 / WSCALE

_CACHE: dict = {}


def _build(flags):
    has_bq, has_bk, has_bv, has_outb, has_embb = flags
    from contextlib import ExitStack
    import concourse.bass as bass
    import concourse.bacc as bacc
    import concourse.tile as tile
    import concourse.mybir as mybir
    from concourse.masks import make_identity

    f32 = mybir.dt.float32
    bf16 = mybir.dt.bfloat16
    f8 = mybir.dt.float8e4
    Alu = mybir.AluOpType
    Act = mybir.ActivationFunctionType
    DR = mybir.MatmulPerfMode.DoubleRow

    nc = bacc.Bacc("TRN2", target_bir_lowering=False, debug=False,
                   enable_asserts=True, num_devices=NCORES)

    x_in = nc.declare_dram_parameter("x", [TH, D], f32, isOutput=False)
    mk_in = nc.declare_dram_parameter("mask", [TH], f32, isOutput=False)
    emb_in = nc.declare_dram_parameter("embv", [TEH], f32, isOutput=False)
    wq_in = nc.declare_dram_parameter("wq", [KC, P, D], f8, isOutput=False)
    wk_in = nc.declare_dram_parameter("wk", [KC, P, D], f8, isOutput=False)
    wv_in = nc.declare_dram_parameter("wv", [KC, P, D], f8, isOutput=False)
    wo_in = nc.declare_dram_parameter("wo", [KC, P, D], f8, isOutput=False)
    we_in = nc.declare_dram_parameter("we", [TEC, P, 2 * D], bf16, isOutput=False)
    vec_in = nc.declare_dram_parameter("vecs", [1, 8, D], f32, isOutput=False)
    y_out = nc.declare_dram_parameter("y", [TH, D], f32, isOutput=True)

    PAIRS = [[0, 1], [2, 3], [4, 5], [6, 7]]

    with tile.TileContext(nc) as tc, ExitStack() as ctx:
        const = ctx.enter_context(tc.tile_pool(name="const", bufs=1))
        wpool = ctx.enter_context(tc.tile_pool(name="wpool", bufs=1))
        xstash = ctx.enter_context(tc.tile_pool(name="xstash", bufs=NT))
        qstash = ctx.enter_context(tc.tile_pool(name="qstash", bufs=NT))
        ystash = ctx.enter_context(tc.tile_pool(name="ystash", bufs=NT))
        stat = ctx.enter_context(tc.tile_pool(name="stat", bufs=1))
        dramp = ctx.enter_context(tc.tile_pool(name="dram", bufs=1, space="DRAM"))

        ident = const.tile([P, P], bf16)
        make_identity(nc, ident)
        eps_t = const.tile([P, 1], f32)
        nc.vector.memset(eps_t, EPS)
        ones8 = const.tile([P, H, 1], bf16)
        nc.vector.memset(ones8, 1.0)
        ones_row = const.tile([1, P], bf16)
        nc.vector.memset(ones_row, 1.0)
        one_f32 = const.tile([1, 1], f32)
        nc.vector.memset(one_f32, 1.0)

        wq_s = wpool.tile([P, KC, D], f8)
        nc.sync.dma_start(out=wq_s, in_=wq_in[:].rearrange("c p d -> p c d"))
        wk_s = wpool.tile([P, KC, D], f8)
        nc.sync.dma_start(out=wk_s, in_=wk_in[:].rearrange("c p d -> p c d"))
        wv_s = wpool.tile([P, KC, D], f8)
        nc.sync.dma_start(out=wv_s, in_=wv_in[:].rearrange("c p d -> p c d"))
        wo_s = wpool.tile([P, KC, D], f8)
        nc.sync.dma_start(out=wo_s, in_=wo_in[:].rearrange("c p d -> p c d"))
        we_s = wpool.tile([P, TEC, 2 * D], bf16)
        nc.sync.dma_start(out=we_s, in_=we_in[:].rearrange("c p d -> p c d"))
        mask_s = wpool.tile([P, NT], f32)
        nc.sync.dma_start(out=mask_s, in_=mk_in[:].rearrange("(n p) -> p n", p=P))
        vec_s = wpool.tile([1, 8, D], f32)
        nc.sync.dma_start(out=vec_s, in_=vec_in[:])

        mv_st = stat.tile([P, NT, 2], f32)
        s1_st = stat.tile([P, NT], f32)
        s2_st = stat.tile([P, NT], f32)
        rq_st = stat.tile([P, NT, H, 1], f32)
        rstd_a = stat.tile([P, NT], f32)
        nb_a = stat.tile([P, NT], f32)
        sd_a = stat.tile([P, NT], f32)
        rstd2_a = stat.tile([P, NT], f32)
        nb2_a = stat.tile([P, NT], f32)
        sd2_a = stat.tile([P, NT], f32)

        cc_in_a = dramp.tile([CCN], bf16)
        cc_out_a = dramp.tile([CCN], bf16)
        cc_in_b = dramp.tile([CCU], bf16)
        cc_out_b = dramp.tile([CCU], bf16)

        x_tiles = []
        q_tiles = []
        y_tiles = []

        with ExitStack() as ctxA:
            work = ctxA.enter_context(tc.tile_pool(name="work", bufs=3))
            psP = ctxA.enter_context(tc.tile_pool(name="psP", bufs=1, space="PSUM"))
            psT = ctxA.enter_context(tc.tile_pool(name="psT", bufs=2, space="PSUM"))
            psU = ctxA.enter_context(tc.tile_pool(name="psU", bufs=1, space="PSUM"))
            embp = ctxA.enter_context(tc.tile_pool(name="embp", bufs=1))

            # ---- bias broadcast tiles (only when biases nonzero) ----
            def bcast_row(row_idx, name):
                pb = psT.tile([P, KC, P], f32, tag="pT")
                rbf = const.tile([1, D], bf16, tag="rbf_" + name)
                nc.vector.tensor_copy(out=rbf, in_=vec_s[:, row_idx, :])
                nc.tensor.matmul(out=pb[:].rearrange("p a b -> p (a b)"),
                                 lhsT=ones_row, rhs=rbf, start=True, stop=True)
                bc = const.tile([P, D], f32, tag="bc_" + name)
                nc.scalar.copy(out=bc, in_=pb[:].rearrange("p a b -> p (a b)"))
                return bc

            bq_bc = bcast_row(0, "bq") if has_bq else None
            bk_bc = bcast_row(1, "bk") if has_bk else None
            bv_bc = bcast_row(2, "bv") if has_bv else None
            ob_bc = bcast_row(3, "ob") if has_outb else None

            # ---- emb projection partial (this core's TE shard) ----
            # silu via exp-table ops only: e*sigmoid(e) = e/(1+exp(-e))
            embt = embp.tile([P, TEC], f32)
            nc.sync.dma_start(out=embt, in_=emb_in[:].rearrange("(c p) -> p c", p=P))
            emneg = embp.tile([P, TEC], f32)
            nc.scalar.activation(out=emneg, in_=embt, func=Act.Exp, scale=-1.0)
            nc.vector.tensor_scalar_add(out=emneg, in0=emneg, scalar1=1.0)
            nc.vector.reciprocal(out=emneg, in_=emneg)
            embs = embp.tile([P, TEC], bf16)
            nc.vector.tensor_mul(out=embs, in0=embt, in1=emneg)
            pe0 = psP.tile([P, D], f32, tag="pq")
            pe1 = psP.tile([P, D], f32, tag="pk")
            for j in range(TEC):
                nc.tensor.matmul(out=pe0[0:1, :], lhsT=embs[:, j:j + 1],
                                 rhs=we_s[:, j, 0:D],
                                 start=(j == 0), stop=(j == TEC - 1))
            for j in range(TEC):
                nc.tensor.matmul(out=pe1[0:1, :], lhsT=embs[:, j:j + 1],
                                 rhs=we_s[:, j, D:2 * D],
                                 start=(j == 0), stop=(j == TEC - 1))
            emb_part = embp.tile([1, 2 * D], bf16)
            nc.scalar.copy(out=emb_part[:, 0:D], in_=pe0[0:1, :])
            nc.scalar.copy(out=emb_part[:, D:2 * D], in_=pe1[0:1, :])

            u0 = psU.tile([P, 2, 2 * (Dh + 1)], f32, tag="u0")
            u1 = psU.tile([P, 2, 2 * (Dh + 1)], f32, tag="u1")
            usb = [embp.tile([64, H, Dh + 1], bf16, tag=f"usb{g}",
                             name=f"usb{g}")
                   for g in range(2)]

            def stats_tile(i):
                xt = xstash.tile([P, D], f32, tag="x")
                x_tiles.append(xt)
                nc.sync.dma_start(out=xt, in_=x_in[i * P:(i + 1) * P, :])
                st = work.tile([P, 6], f32, tag="st")
                nc.vector.bn_stats(out=st, in_=xt)
                nc.vector.bn_aggr(out=mv_st[:, i, :], in_=st)

            def stats_batch(g):
                sl = slice(g * NG, (g + 1) * NG)
                nc.scalar.activation(out=sd_a[:, sl], in_=mv_st[:, sl, 1],
                                     func=Act.Ln, bias=eps_t)
                nc.scalar.activation(out=rstd_a[:, sl], in_=sd_a[:, sl],
                                     func=Act.Exp, scale=-0.5)
                nc.vector.tensor_mul(out=nb_a[:, sl], in0=mv_st[:, sl, 0],
                                     in1=rstd_a[:, sl])
                nc.vector.tensor_scalar_mul(out=nb_a[:, sl], in0=nb_a[:, sl],
                                            scalar1=-1.0)

            pair_state = {}

            def proj_tile(i):
                # normalize on gpsimd (scalar tables untouched)
                xn = work.tile([P, D], bf16, tag="xn")
                nc.gpsimd.tensor_scalar(out=xn, in0=x_tiles[i],
                                        scalar1=rstd_a[:, i:i + 1],
                                        scalar2=nb_a[:, i:i + 1],
                                        op0=Alu.mult, op1=Alu.add)
                xTp = psT.tile([P, KC, P], bf16, tag="pT")
                for j in range(KC):
                    nc.tensor.transpose(xTp[:, j, :], xn[:, j * P:(j + 1) * P],
                                        ident)
                xT = work.tile([P, KC, P], f8, tag="xT")
                for j in range(KC):
                    if j % 2 == 0:
                        nc.scalar.copy(out=xT[:, j, :], in_=xTp[:, j, :])
                    else:
                        nc.vector.tensor_copy(out=xT[:, j, :], in_=xTp[:, j, :])

                pq = psP.tile([P, D], f32, tag="pq")
                pk = psP.tile([P, D], f32, tag="pk")
                pv = psP.tile([P, D], f32, tag="pv")
                for m in range(2):
                    nc.tensor.matmul(out=pq, lhsT=xT[:, 2 * m:2 * m + 2, :],
                                     rhs=wq_s[:, 2 * m:2 * m + 2, :],
                                     start=(m == 0), stop=(m == 1),
                                     perf_mode=DR)
                for m in range(2):
                    nc.tensor.matmul(out=pk, lhsT=xT[:, 2 * m:2 * m + 2, :],
                                     rhs=wk_s[:, 2 * m:2 * m + 2, :],
                                     start=(m == 0), stop=(m == 1),
                                     perf_mode=DR)
                for m in range(2):
                    nc.tensor.matmul(out=pv, lhsT=xT[:, 2 * m:2 * m + 2, :],
                                     rhs=wv_s[:, 2 * m:2 * m + 2, :],
                                     start=(m == 0), stop=(m == 1),
                                     perf_mode=DR)
                if has_bq:
                    nc.vector.scalar_tensor_tensor(
                        out=pq, in0=pq, scalar=RWS, in1=bq_bc,
                        op0=Alu.mult, op1=Alu.add)
                if has_bk:
                    nc.vector.scalar_tensor_tensor(
                        out=pk, in0=pk, scalar=RWS, in1=bk_bc,
                        op0=Alu.mult, op1=Alu.add)
                if has_bv:
                    nc.vector.scalar_tensor_tensor(
                        out=pv, in0=pv, scalar=RWS, in1=bv_bc,
                        op0=Alu.mult, op1=Alu.add)
                qsc = 1.0 if has_bq else RWS
                ksc = 1.0 if has_bk else RWS
                vsc = 1.0 if has_bv else RWS

                # q: exp, per-head sums, PE transpose into qT stash
                qt = work.tile([P, D], bf16, tag="qt")
                nc.scalar.activation(out=qt, in_=pq, func=Act.Exp, scale=qsc)
                qs = work.tile([P, H, 1], f32, tag="qs")
                nc.vector.reduce_sum(
                    out=qs, in_=qt[:].rearrange("p (h d) -> p h d", h=H),
                    axis=mybir.AxisListType.X)
                nc.vector.reciprocal(out=rq_st[:, i, :, :], in_=qs)
                qTp = psT.tile([P, KC, P], bf16, tag="pT")
                for j in range(KC):
                    nc.tensor.transpose(qTp[:, j, :], qt[:, j * P:(j + 1) * P],
                                        ident)
                qT = qstash.tile([P, KC, P], bf16, tag="qT")
                q_tiles.append(qT)
                for j in range(KC):
                    if j % 2 == 0:
                        nc.scalar.copy(out=qT[:, j, :], in_=qTp[:, j, :])
                    else:
                        nc.vector.tensor_copy(out=qT[:, j, :], in_=qTp[:, j, :])

                # k, v in fp8 for the DoubleRow U einsum over tile pairs
                if i % 2 == 0:
                    et2 = work.tile([P, 2, D], f8, tag="et2")
                    va2 = work.tile([P, 2, H, Dh + 1], f8, tag="va2")
                    pair_state["et2"] = et2
                    pair_state["va2"] = va2
                else:
                    et2 = pair_state["et2"]
                    va2 = pair_state["va2"]
                nc.scalar.activation(out=et2[:, i % 2, :], in_=pk,
                                     func=Act.Exp, scale=ksc)
                nc.vector.tensor_scalar(
                    out=va2[:, i % 2, :, 0:Dh],
                    in0=pv[:].rearrange("p (h d) -> p h d", h=H),
                    scalar1=mask_s[:, i:i + 1], scalar2=vsc,
                    op0=Alu.mult, op1=Alu.mult)
                nc.gpsimd.tensor_scalar_mul(out=va2[:, i % 2, :, Dh:Dh + 1],
                                            in0=ones8,
                                            scalar1=mask_s[:, i:i + 1])
                if i % 2 == 1:
                    ig = (i // 2) % (NG // 2)   # pair index within group
                    for p2 in range(4):
                        u = u0 if p2 < 2 else u1
                        nc.tensor.matmul(
                            out=u[:, p2 % 2, :],
                            lhsT=et2[:, :, p2 * P:(p2 + 1) * P],
                            rhs=va2[:, :, 2 * p2:2 * p2 + 2, :].rearrange(
                                "p k h f -> p k (h f)"),
                            start=(ig == 0 and p2 % 2 == 0),
                            stop=(ig == NG // 2 - 1 and p2 % 2 == 1),
                            perf_mode=DR)

            def ship_u(g):
                u_sb = usb[g]
                for p2 in range(4):
                    u = u0 if p2 < 2 else u1
                    nc.scalar.copy(out=u_sb[:, 2 * p2, :],
                                   in_=u[0:64, p2 % 2, 0:Dh + 1])
                    nc.scalar.copy(out=u_sb[:, 2 * p2 + 1, :],
                                   in_=u[64:P, p2 % 2, Dh + 1:2 * Dh + 2])
                cc_in = cc_in_a if g == 0 else cc_in_b
                cc_out = cc_out_a if g == 0 else cc_out_b
                nc.sync.dma_start(
                    out=cc_in[0:CCU].rearrange("(p h f) -> p h f", p=64, h=H),
                    in_=u_sb)
                if g == 0:
                    nc.sync.dma_start(
                        out=cc_in[CCU:CCN].rearrange("(a f) -> a f", a=1),
                        in_=emb_part)
                nc.gpsimd.collective_compute(
                    "AllReduce", Alu.add, replica_groups=PAIRS,
                    ins=[cc_in[:]], outs=[cc_out[:]])

            # group 0 stats -> batch0 -> [group 1 stats || group 0 proj]
            # -> AR(U half 0) -> batch1 -> group 1 proj -> AR(U half 1)
            for i in range(NG):
                stats_tile(i)
            stats_batch(0)
            for i in range(NG):
                stats_tile(NG + i)
                proj_tile(i)
            ship_u(0)
            stats_batch(1)
            for i in range(NG, NT):
                proj_tile(i)
            ship_u(1)

        # ---- phase B prologue: attn state + stylization vectors ----
        with ExitStack() as ctxB:
            workB = ctxB.enter_context(tc.tile_pool(name="workB", bufs=3))
            psB = ctxB.enter_context(tc.tile_pool(name="psB", bufs=2, space="PSUM"))
            embB = ctxB.enter_context(tc.tile_pool(name="embB", bufs=1))

            u_fa = embB.tile([P, H, Dh + 1], bf16)
            nc.sync.dma_start(
                out=u_fa[0:64], in_=cc_out_a[0:CCU].rearrange(
                    "(p h f) -> p h f", p=64, h=H))
            nc.sync.dma_start(
                out=u_fa[64:P], in_=cc_out_a[0:CCU].rearrange(
                    "(p h f) -> p h f", p=64, h=H))
            u_fb = embB.tile([P, H, Dh + 1], bf16)
            nc.sync.dma_start(
                out=u_fb[0:64], in_=cc_out_b[0:CCU].rearrange(
                    "(p h f) -> p h f", p=64, h=H))
            nc.sync.dma_start(
                out=u_fb[64:P], in_=cc_out_b[0:CCU].rearrange(
                    "(p h f) -> p h f", p=64, h=H))
            u_f = embB.tile([P, H, Dh + 1], f32)
            nc.vector.tensor_add(out=u_f, in0=u_fa, in1=u_fb)
            emb_f = embB.tile([1, 2 * D], bf16)
            nc.sync.dma_start(
                out=emb_f, in_=cc_out_a[CCU:CCN].rearrange("(a f) -> a f", a=1))

            rs = embB.tile([P, H, 1], f32)
            nc.vector.reciprocal(out=rs, in_=u_f[:, :, Dh:Dh + 1])
            attn2 = embB.tile([P, KC, P], bf16)
            nc.gpsimd.memset(attn2, 0.0)
            for h in range(H):
                base = 64 * (h % 2)
                nc.vector.tensor_scalar_mul(
                    out=attn2[base:base + 64, h // 2, base:base + 64],
                    in0=u_f[base:base + 64, h, 0:Dh],
                    scalar1=rs[base:base + 64, h, :])

            srow = embB.tile([1, D], f32)
            shrow = embB.tile([1, D], f32)
            if has_embb:
                nc.vector.tensor_add(out=srow, in0=emb_f[:, 0:D],
                                     in1=vec_s[:, 6, :])
                nc.vector.tensor_add(out=shrow, in0=emb_f[:, D:2 * D],
                                     in1=vec_s[:, 7, :])
            else:
                nc.vector.tensor_copy(out=srow, in_=emb_f[:, 0:D])
                nc.vector.tensor_copy(out=shrow, in_=emb_f[:, D:2 * D])
            t1 = embB.tile([1, D], f32)
            nc.vector.tensor_scalar_add(out=t1, in0=srow, scalar1=1.0)
            arow = embB.tile([1, D], f32)
            nc.vector.tensor_mul(out=arow, in0=t1, in1=vec_s[:, 4, :])
            crow = embB.tile([1, D], f32)
            nc.vector.tensor_mul(out=crow, in0=t1, in1=vec_s[:, 5, :])
            nc.vector.tensor_add(out=crow, in0=crow, in1=shrow)

            # transpose a,c rows to per-chunk columns [P, KC]
            acp = psB.tile([P, 2, KC], f32, tag="ac", bufs=1)
            for j in range(KC):
                nc.tensor.transpose(acp[:, 0, j:j + 1],
                                    arow[:, j * P:(j + 1) * P],
                                    one_f32)
                nc.tensor.transpose(acp[:, 1, j:j + 1],
                                    crow[:, j * P:(j + 1) * P],
                                    one_f32)
            a_col = embB.tile([P, KC], f32)
            nc.scalar.copy(out=a_col, in_=acp[:, 0, :])
            c_col = embB.tile([P, KC], f32)
            nc.scalar.copy(out=c_col, in_=acp[:, 1, :])

            # ---- B1: y = q@attn, broadcast 1/S, LN2 stats ----
            for i in range(NT):
                py = psB.tile([P, KC, P], f32, tag="py")
                for j in range(KC):
                    nc.tensor.matmul(out=py[:, j, :], lhsT=q_tiles[i][:, j, :],
                                     rhs=attn2[:, j, :], start=True, stop=True)
                ysb = ystash.tile([P, D], bf16, tag="ysb")
                y_tiles.append(ysb)
                nc.vector.tensor_mul(
                    out=ysb[:].rearrange("p (h d) -> p h d", h=H),
                    in0=py[:].rearrange("p a b -> p (a b)").rearrange(
                        "p (h d) -> p h d", h=H),
                    in1=rq_st[:, i, :, :].to_broadcast([P, H, Dh]))
                # LN2 sums via ACT accumulate (scalar engine is idle here)
                dumm = workB.tile([P, D], bf16, tag="dumm")
                nc.scalar.activation(out=dumm, in_=ysb, func=Act.Identity,
                                     accum_out=s1_st[:, i:i + 1])
                nc.scalar.activation(out=dumm, in_=ysb, func=Act.Square,
                                     accum_out=s2_st[:, i:i + 1])

            # batched LN2 rstd/bias: var = E[y^2] - mu^2
            mu2_a = stat.tile([P, NT], f32)
            nc.vector.tensor_scalar_mul(out=mu2_a, in0=s1_st, scalar1=1.0 / D)
            var2_a = stat.tile([P, NT], f32)
            nc.vector.tensor_scalar_mul(out=var2_a, in0=s2_st, scalar1=1.0 / D)
            musq = stat.tile([P, NT], f32)
            nc.vector.tensor_mul(out=musq, in0=mu2_a, in1=mu2_a)
            nc.vector.tensor_sub(out=var2_a, in0=var2_a, in1=musq)
            nc.scalar.activation(out=sd2_a, in_=var2_a, func=Act.Ln,
                                 bias=eps_t)
            nc.scalar.activation(out=rstd2_a, in_=sd2_a, func=Act.Exp,
                                 scale=-0.5)
            nc.vector.tensor_mul(out=nb2_a, in0=mu2_a, in1=rstd2_a)
            nc.vector.tensor_scalar_mul(out=nb2_a, in0=nb2_a, scalar1=-1.0)

            # ---- B2: normalize, transpose, fused stylize+silu, out proj ----
            for i in range(NT):
                z = workB.tile([P, D], bf16, tag="z")
                nc.vector.tensor_scalar(out=z, in0=y_tiles[i],
                                        scalar1=rstd2_a[:, i:i + 1],
                                        scalar2=nb2_a[:, i:i + 1],
                                        op0=Alu.mult, op1=Alu.add)
                zTp = psB.tile([P, KC, P], bf16, tag="pT")
                for j in range(KC):
                    nc.tensor.transpose(zTp[:, j, :], z[:, j * P:(j + 1) * P],
                                        ident)
                # silu(zT*a + c) per chunk: a,c are per-partition here
                hT = workB.tile([P, KC, P], f8, tag="hT")
                for j in range(KC):
                    nc.scalar.activation(out=hT[:, j, :], in_=zTp[:, j, :],
                                         func=Act.Silu,
                                         scale=a_col[:, j:j + 1],
                                         bias=c_col[:, j:j + 1])
                po = psB.tile([P, D], f32, tag="po")
                for m in range(2):
                    nc.tensor.matmul(out=po, lhsT=hT[:, 2 * m:2 * m + 2, :],
                                     rhs=wo_s[:, 2 * m:2 * m + 2, :],
                                     start=(m == 0), stop=(m == 1),
                                     perf_mode=DR)
                osb = workB.tile([P, D], f32, tag="osb")
                nc.vector.scalar_tensor_tensor(
                    out=osb, in0=po, scalar=RWS, in1=x_tiles[i],
                    op0=Alu.mult, op1=Alu.add)
                if has_outb:
                    nc.vector.tensor_add(out=osb, in0=osb, in1=ob_bc)
                nc.sync.dma_start(out=y_out[i * P:(i + 1) * P, :], in_=osb)

    nc.compile()
    return nc


def _to_f8(a):
    return np.clip(a * WSCALE, -240.0, 240.0).astype(ml_dtypes.float8_e4m3fn)


def _prep(inputs, flags):
    bf = ml_dtypes.bfloat16
    x = np.asarray(inputs["x"], np.float32)
    emb = np.asarray(inputs["emb"], np.float32)
    src_mask = np.asarray(inputs["src_mask"], np.float32)
    gamma = np.asarray(inputs["gamma"], np.float32)
    beta = np.asarray(inputs["beta"], np.float32)
    gamma2 = np.asarray(inputs["gamma2"], np.float32)
    beta2 = np.asarray(inputs["beta2"], np.float32)
    emb_b = np.asarray(inputs["emb_b"], np.float32)
    out_b = np.asarray(inputs["out_b"], np.float32)

    def foldW(Wname):
        W = np.asarray(inputs[Wname], np.float32)
        return np.ascontiguousarray(_to_f8(gamma[:, None] * W).reshape(KC, P, D))

    wq, wk, wv = foldW("Wq"), foldW("Wk"), foldW("Wv")
    wo = np.ascontiguousarray(
        _to_f8(np.asarray(inputs["out_W"], np.float32)).reshape(KC, P, D))
    bq_f = np.asarray(inputs["bq"], np.float32) + beta @ np.asarray(inputs["Wq"], np.float32)
    bk_f = np.asarray(inputs["bk"], np.float32) + beta @ np.asarray(inputs["Wk"], np.float32)
    bv_f = np.asarray(inputs["bv"], np.float32) + beta @ np.asarray(inputs["Wv"], np.float32)
    vecs = np.ascontiguousarray(np.stack(
        [bq_f, bk_f, bv_f, out_b, gamma2, beta2, emb_b[:D], emb_b[D:]]
    ).astype(np.float32).reshape(1, 8, D))
    emb_W = np.asarray(inputs["emb_W"], np.float32)
    we_halves = [
        np.ascontiguousarray(
            emb_W[t * TEH:(t + 1) * TEH].astype(bf).reshape(TEC, P, 2 * D))
        for t in range(2)]

    in_maps = []
    for c in range(NCORES):
        b, th = c // 2, c % 2
        sl = slice(th * TH, (th + 1) * TH)
        in_maps.append({
            "x": np.ascontiguousarray(x[b, sl]),
            "mask": np.ascontiguousarray(src_mask[b, sl, 0]),
            "embv": np.ascontiguousarray(emb[b, th * TEH:(th + 1) * TEH]),
            "wq": wq, "wk": wk, "wv": wv, "wo": wo,
            "we": we_halves[th],
            "vecs": vecs,
        })
    return in_maps


def _flags(inputs):
    beta = np.asarray(inputs["beta"], np.float32)

    def nz(v):
        return bool(np.any(np.asarray(v) != 0))

    bq_f = np.asarray(inputs["bq"], np.float32) + beta @ np.asarray(inputs["Wq"], np.float32)
    bk_f = np.asarray(inputs["bk"], np.float32) + beta @ np.asarray(inputs["Wk"], np.float32)
    bv_f = np.asarray(inputs["bv"], np.float32) + beta @ np.asarray(inputs["Wv"], np.float32)
    return (nz(bq_f), nz(bk_f), nz(bv_f), nz(inputs["out_b"]), nz(inputs["emb_b"]))


def get_nc_and_inmaps(**inputs):
    flags = _flags(inputs)
    if flags not in _CACHE:
        _CACHE[flags] = _build(flags)
    return _CACHE[flags], _prep(inputs, flags)


def kernel(**inputs):
    from concourse.bass_utils import run_bass_kernel_spmd
    nc, in_maps = get_nc_and_inmaps(**inputs)
    res = run_bass_kernel_spmd(nc, in_maps, list(range(NCORES)))
    out = np.empty((B, T, D), np.float32)
    for c in range(NCORES):
        b, th = c // 2, c % 2
        out[b, th * TH:(th + 1) * TH] = res.results[c]["y"]
    return out


# revision 20
# speedup vs baseline: 2.5386x; 1.0335x over previous
"""Trainium2 Bass kernel for nn_LinearTemporalSelfAttention (B=4,T=8192,D=512,H=8).

Sharding: 8 cores = B(4) x T-halves(2). Each core owns a (b, t-half) slab
(4096 x 512) end-to-end. Cross-core data is only the KV-state einsum
(sum over full T) and the emb projection (emb_W sharded over TE within a
pair) — folded into two pair-wise bf16 AllReduces (first half of U mid-
phase-A so it overlaps compute, second half at the end).

v3 structure:
 - LN rstd batched per 16-tile group (one Ln + one Exp each) so the
   scalar engine never thrashes activation tables; groups interleave
   with the projection loop so stats DMA/vector work overlaps PE work.
 - All transposes on the PE (matmul is_transpose) + PSUM evac.
 - QKV / out projections and the KV-state einsum run fp8(e4m3)
   DoubleRow (2 k-subtiles per pass). Weights x64 on host; exp(k),
   masked v quantized to fp8 (their errors average out over T in U).
 - Per-head softmax-q 1/S applied in ONE broadcast tensor_tensor op.
 - Phase B pass 2 works in transposed space: PE-transpose LN2-normalized
   y, then a single ACT Silu per chunk applies the stylization scale
   and shift (per-partition there) while evacuating PSUM into fp8.
 - Elementwise work spread across scalar/vector/gpsimd.
"""
import numpy as np
import ml_dtypes

B, T, D, H, TE = 4, 8192, 512, 8, 2048
Dh = D // H          # 64
EPS = 1e-5
NCORES = 8
TH = T // 2          # 4096 rows per core
P = 128
NT = TH // P         # 32 row tiles
NG = 16              # tiles per stat/U group
KC = D // P          # 4 contraction chunks
TEH = TE // 2        # 1024 te rows per core
TEC = TEH // P       # 8 te chunks
CCU = 64 * H * (Dh + 1)     # 33280 floats of U_aug
CCN = CCU + 2 * D           # + emb partial
WSCALE = 64.0        # fp8 weight prescale
RWS = 1.0 / WSCALE

_CACHE: dict = {}


def _build(flags):
    has_bq, has_bk, has_bv, has_outb, has_embb = flags
    from contextlib import ExitStack
    import concourse.bass as bass
    import concourse.bacc as bacc
    import concourse.tile as tile
    import concourse.mybir as mybir
    from concourse.masks import make_identity

    f32 = mybir.dt.float32
    bf16 = mybir.dt.bfloat16
    f8 = mybir.dt.float8e4
    Alu = mybir.AluOpType
    Act = mybir.ActivationFunctionType
    DR = mybir.MatmulPerfMode.DoubleRow

    nc = bacc.Bacc("TRN2", target_bir_lowering=False, debug=False,
                   enable_asserts=True, num_devices=NCORES)

    x_in = nc.declare_dram_parameter("x", [TH, D], f32, isOutput=False)
    mk_in = nc.declare_dram_parameter("mask", [TH], f32, isOutput=False)
    emb_in = nc.declare_dram_parameter("embv", [TEH], f32, isOutput=False)
    wq_in = nc.declare_dram_parameter("wq", [KC, P, D], f8, isOutput=False)
    wk_in = nc.declare_dram_parameter("wk", [KC, P, D], f8, isOutput=False)
    wv_in = nc.declare_dram_parameter("wv", [KC, P, D], f8, isOutput=False)
    wo_in = nc.declare_dram_parameter("wo", [KC, P, D], bf16, isOutput=False)
    we_in = nc.declare_dram_parameter("we", [TEC, P, 2 * D], bf16, isOutput=False)
    vec_in = nc.declare_dram_parameter("vecs", [1, 8, D], f32, isOutput=False)
    y_out = nc.declare_dram_parameter("y", [TH, D], f32, isOutput=True)

    PAIRS = [[0, 1], [2, 3], [4, 5], [6, 7]]

    with tile.TileContext(nc) as tc, ExitStack() as ctx:
        const = ctx.enter_context(tc.tile_pool(name="const", bufs=1))
        wpool = ctx.enter_context(tc.tile_pool(name="wpool", bufs=1))
        xstash = ctx.enter_context(tc.tile_pool(name="xstash", bufs=NT))
        qstash = ctx.enter_context(tc.tile_pool(name="qstash", bufs=NT))
        ystash = ctx.enter_context(tc.tile_pool(name="ystash", bufs=NT))
        stat = ctx.enter_context(tc.tile_pool(name="stat", bufs=1))
        dramp = ctx.enter_context(tc.tile_pool(name="dram", bufs=1, space="DRAM"))

        ident = const.tile([P, P], bf16)
        make_identity(nc, ident)
        eps_t = const.tile([P, 1], f32)
        nc.vector.memset(eps_t, EPS)
        ones8 = const.tile([P, H, 1], bf16)
        nc.vector.memset(ones8, 1.0)
        ones_row = const.tile([1, P], bf16)
        nc.vector.memset(ones_row, 1.0)
        one_f32 = const.tile([1, 1], f32)
        nc.vector.memset(one_f32, 1.0)

        wq_s = wpool.tile([P, KC, D], f8)
        nc.sync.dma_start(out=wq_s, in_=wq_in[:].rearrange("c p d -> p c d"))
        wk_s = wpool.tile([P, KC, D], f8)
        nc.sync.dma_start(out=wk_s, in_=wk_in[:].rearrange("c p d -> p c d"))
        wv_s = wpool.tile([P, KC, D], f8)
        nc.sync.dma_start(out=wv_s, in_=wv_in[:].rearrange("c p d -> p c d"))
        wo_s = wpool.tile([P, KC, D], bf16)
        nc.sync.dma_start(out=wo_s, in_=wo_in[:].rearrange("c p d -> p c d"))
        we_s = wpool.tile([P, TEC, 2 * D], bf16)
        nc.sync.dma_start(out=we_s, in_=we_in[:].rearrange("c p d -> p c d"))
        mask_s = wpool.tile([P, NT], f32)
        nc.sync.dma_start(out=mask_s, in_=mk_in[:].rearrange("(n p) -> p n", p=P))
        vec_s = wpool.tile([1, 8, D], f32)
        nc.sync.dma_start(out=vec_s, in_=vec_in[:])

        mv_st = stat.tile([P, NT, 2], f32)
        s1_st = stat.tile([P, NT], f32)
        s2_st = stat.tile([P, NT], f32)
        rq_st = stat.tile([P, NT, H, 1], f32)
        rstd_a = stat.tile([P, NT], f32)
        nb_a = stat.tile([P, NT], f32)
        sd_a = stat.tile([P, NT], f32)
        rstd2_a = stat.tile([P, NT], f32)
        nb2_a = stat.tile([P, NT], f32)
        sd2_a = stat.tile([P, NT], f32)

        cc_in_a = dramp.tile([CCN], bf16)
        cc_out_a = dramp.tile([CCN], bf16)
        cc_in_b = dramp.tile([CCU], bf16)
        cc_out_b = dramp.tile([CCU], bf16)

        x_tiles = []
        q_tiles = []
        y_tiles = []

        with ExitStack() as ctxA:
            work = ctxA.enter_context(tc.tile_pool(name="work", bufs=3))
            psP = ctxA.enter_context(tc.tile_pool(name="psP", bufs=1, space="PSUM"))
            psT = ctxA.enter_context(tc.tile_pool(name="psT", bufs=2, space="PSUM"))
            psU = ctxA.enter_context(tc.tile_pool(name="psU", bufs=1, space="PSUM"))
            embp = ctxA.enter_context(tc.tile_pool(name="embp", bufs=1))

            # ---- bias broadcast tiles (only when biases nonzero) ----
            def bcast_row(row_idx, name):
                pb = psT.tile([P, KC, P], f32, tag="pT")
                rbf = const.tile([1, D], bf16, tag="rbf_" + name)
                nc.vector.tensor_copy(out=rbf, in_=vec_s[:, row_idx, :])
                nc.tensor.matmul(out=pb[:].rearrange("p a b -> p (a b)"),
                                 lhsT=ones_row, rhs=rbf, start=True, stop=True)
                bc = const.tile([P, D], f32, tag="bc_" + name)
                nc.scalar.copy(out=bc, in_=pb[:].rearrange("p a b -> p (a b)"))
                return bc

            bq_bc = bcast_row(0, "bq") if has_bq else None
            bk_bc = bcast_row(1, "bk") if has_bk else None
            bv_bc = bcast_row(2, "bv") if has_bv else None
            ob_bc = bcast_row(3, "ob") if has_outb else None

            # ---- emb projection partial (this core's TE shard) ----
            # silu via exp-table ops only: e*sigmoid(e) = e/(1+exp(-e))
            embt = embp.tile([P, TEC], f32)
            nc.sync.dma_start(out=embt, in_=emb_in[:].rearrange("(c p) -> p c", p=P))
            emneg = embp.tile([P, TEC], f32)
            nc.scalar.activation(out=emneg, in_=embt, func=Act.Exp, scale=-1.0)
            nc.vector.tensor_scalar_add(out=emneg, in0=emneg, scalar1=1.0)
            nc.vector.reciprocal(out=emneg, in_=emneg)
            embs = embp.tile([P, TEC], bf16)
            nc.vector.tensor_mul(out=embs, in0=embt, in1=emneg)
            pe0 = psP.tile([P, D], f32, tag="pq")
            pe1 = psP.tile([P, D], f32, tag="pk")
            for j in range(TEC):
                nc.tensor.matmul(out=pe0[0:1, :], lhsT=embs[:, j:j + 1],
                                 rhs=we_s[:, j, 0:D],
                                 start=(j == 0), stop=(j == TEC - 1))
            for j in range(TEC):
                nc.tensor.matmul(out=pe1[0:1, :], lhsT=embs[:, j:j + 1],
                                 rhs=we_s[:, j, D:2 * D],
                                 start=(j == 0), stop=(j == TEC - 1))
            emb_part = embp.tile([1, 2 * D], bf16)
            nc.scalar.copy(out=emb_part[:, 0:D], in_=pe0[0:1, :])
            nc.scalar.copy(out=emb_part[:, D:2 * D], in_=pe1[0:1, :])

            u0 = psU.tile([P, 2, 2 * (Dh + 1)], f32, tag="u0")
            u1 = psU.tile([P, 2, 2 * (Dh + 1)], f32, tag="u1")
            usb = [embp.tile([64, H, Dh + 1], bf16, tag=f"usb{g}",
                             name=f"usb{g}")
                   for g in range(2)]

            def stats_tile(i):
                xt = xstash.tile([P, D], f32, tag="x")
                x_tiles.append(xt)
                nc.sync.dma_start(out=xt, in_=x_in[i * P:(i + 1) * P, :])
                st = work.tile([P, 6], f32, tag="st")
                nc.vector.bn_stats(out=st, in_=xt)
                nc.vector.bn_aggr(out=mv_st[:, i, :], in_=st)

            def stats_batch(g):
                sl = slice(g * NG, (g + 1) * NG)
                nc.scalar.activation(out=sd_a[:, sl], in_=mv_st[:, sl, 1],
                                     func=Act.Ln, bias=eps_t)
                nc.scalar.activation(out=rstd_a[:, sl], in_=sd_a[:, sl],
                                     func=Act.Exp, scale=-0.5)
                nc.vector.tensor_mul(out=nb_a[:, sl], in0=mv_st[:, sl, 0],
                                     in1=rstd_a[:, sl])
                nc.vector.tensor_scalar_mul(out=nb_a[:, sl], in0=nb_a[:, sl],
                                            scalar1=-1.0)

            pair_state = {}

            def proj_tile(i):
                # normalize on gpsimd (scalar tables untouched)
                xn = work.tile([P, D], bf16, tag="xn")
                nc.gpsimd.tensor_scalar(out=xn, in0=x_tiles[i],
                                        scalar1=rstd_a[:, i:i + 1],
                                        scalar2=nb_a[:, i:i + 1],
                                        op0=Alu.mult, op1=Alu.add)
                xTp = psT.tile([P, KC, P], bf16, tag="pT")
                for j in range(KC):
                    nc.tensor.transpose(xTp[:, j, :], xn[:, j * P:(j + 1) * P],
                                        ident)
                xT = work.tile([P, KC, P], f8, tag="xT")
                for j in range(KC):
                    if j % 2 == 0:
                        nc.scalar.copy(out=xT[:, j, :], in_=xTp[:, j, :])
                    else:
                        nc.vector.tensor_copy(out=xT[:, j, :], in_=xTp[:, j, :])

                pq = psP.tile([P, D], f32, tag="pq")
                pk = psP.tile([P, D], f32, tag="pk")
                pv = psP.tile([P, D], f32, tag="pv")
                for m in range(2):
                    nc.tensor.matmul(out=pq, lhsT=xT[:, 2 * m:2 * m + 2, :],
                                     rhs=wq_s[:, 2 * m:2 * m + 2, :],
                                     start=(m == 0), stop=(m == 1),
                                     perf_mode=DR)
                for m in range(2):
                    nc.tensor.matmul(out=pk, lhsT=xT[:, 2 * m:2 * m + 2, :],
                                     rhs=wk_s[:, 2 * m:2 * m + 2, :],
                                     start=(m == 0), stop=(m == 1),
                                     perf_mode=DR)
                for m in range(2):
                    nc.tensor.matmul(out=pv, lhsT=xT[:, 2 * m:2 * m + 2, :],
                                     rhs=wv_s[:, 2 * m:2 * m + 2, :],
                                     start=(m == 0), stop=(m == 1),
                                     perf_mode=DR)
                if has_bq:
                    nc.vector.scalar_tensor_tensor(
                        out=pq, in0=pq, scalar=RWS, in1=bq_bc,
                        op0=Alu.mult, op1=Alu.add)
                if has_bk:
                    nc.vector.scalar_tensor_tensor(
                        out=pk, in0=pk, scalar=RWS, in1=bk_bc,
                        op0=Alu.mult, op1=Alu.add)
                if has_bv:
                    nc.vector.scalar_tensor_tensor(
                        out=pv, in0=pv, scalar=RWS, in1=bv_bc,
                        op0=Alu.mult, op1=Alu.add)
                qsc = 1.0 if has_bq else RWS
                ksc = 1.0 if has_bk else RWS
                vsc = 1.0 if has_bv else RWS

                # q: exp, per-head sums, PE transpose into qT stash
                qt = work.tile([P, D], bf16, tag="qt")
                nc.scalar.activation(out=qt, in_=pq, func=Act.Exp, scale=qsc)
                qs = work.tile([P, H, 1], f32, tag="qs")
                nc.vector.reduce_sum(
                    out=qs, in_=qt[:].rearrange("p (h d) -> p h d", h=H),
                    axis=mybir.AxisListType.X)
                nc.vector.reciprocal(out=rq_st[:, i, :, :], in_=qs)
                qTp = psT.tile([P, KC, P], bf16, tag="pT")
                for j in range(KC):
                    nc.tensor.transpose(qTp[:, j, :], qt[:, j * P:(j + 1) * P],
                                        ident)
                qT = qstash.tile([P, KC, P], bf16, tag="qT")
                q_tiles.append(qT)
                for j in range(KC):
                    if j % 2 == 0:
                        nc.scalar.copy(out=qT[:, j, :], in_=qTp[:, j, :])
                    else:
                        nc.vector.tensor_copy(out=qT[:, j, :], in_=qTp[:, j, :])

                # k, v bf16; U einsum per head-pair
                et = work.tile([P, D], bf16, tag="et")
                nc.scalar.activation(out=et, in_=pk, func=Act.Exp, scale=ksc)
                va = work.tile([P, H, Dh + 1], bf16, tag="va")
                nc.vector.tensor_scalar(
                    out=va[:, :, 0:Dh],
                    in0=pv[:].rearrange("p (h d) -> p h d", h=H),
                    scalar1=mask_s[:, i:i + 1], scalar2=vsc,
                    op0=Alu.mult, op1=Alu.mult)
                nc.gpsimd.tensor_scalar_mul(out=va[:, :, Dh:Dh + 1],
                                            in0=ones8,
                                            scalar1=mask_s[:, i:i + 1])
                ig = i % NG
                for p2 in range(4):
                    u = u0 if p2 < 2 else u1
                    nc.tensor.matmul(
                        out=u[:, p2 % 2, :],
                        lhsT=et[:, p2 * P:(p2 + 1) * P],
                        rhs=va[:, 2 * p2:2 * p2 + 2, :].rearrange(
                            "p h f -> p (h f)"),
                        start=(ig == 0 and p2 % 2 == 0),
                        stop=(ig == NG - 1 and p2 % 2 == 1))

            def ship_u(g):
                u_sb = usb[g]
                for p2 in range(4):
                    u = u0 if p2 < 2 else u1
                    nc.scalar.copy(out=u_sb[:, 2 * p2, :],
                                   in_=u[0:64, p2 % 2, 0:Dh + 1])
                    nc.scalar.copy(out=u_sb[:, 2 * p2 + 1, :],
                                   in_=u[64:P, p2 % 2, Dh + 1:2 * Dh + 2])
                cc_in = cc_in_a if g == 0 else cc_in_b
                cc_out = cc_out_a if g == 0 else cc_out_b
                nc.sync.dma_start(
                    out=cc_in[0:CCU].rearrange("(p h f) -> p h f", p=64, h=H),
                    in_=u_sb)
                if g == 0:
                    nc.sync.dma_start(
                        out=cc_in[CCU:CCN].rearrange("(a f) -> a f", a=1),
                        in_=emb_part)
                nc.gpsimd.collective_compute(
                    "AllReduce", Alu.add, replica_groups=PAIRS,
                    ins=[cc_in[:]], outs=[cc_out[:]])

            # group 0 stats -> batch0 -> [group 1 stats || group 0 proj]
            # -> AR(U half 0) -> batch1 -> group 1 proj -> AR(U half 1)
            for i in range(NG):
                stats_tile(i)
            stats_batch(0)
            for i in range(NG):
                stats_tile(NG + i)
                proj_tile(i)
            ship_u(0)
            stats_batch(1)
            for i in range(NG, NT):
                proj_tile(i)
            ship_u(1)

        # ---- phase B prologue: attn state + stylization vectors ----
        with ExitStack() as ctxB:
            workB = ctxB.enter_context(tc.tile_pool(name="workB", bufs=3))
            psB = ctxB.enter_context(tc.tile_pool(name="psB", bufs=2, space="PSUM"))
            embB = ctxB.enter_context(tc.tile_pool(name="embB", bufs=1))

            u_fa = embB.tile([P, H, Dh + 1], bf16)
            nc.sync.dma_start(
                out=u_fa[0:64], in_=cc_out_a[0:CCU].rearrange(
                    "(p h f) -> p h f", p=64, h=H))
            nc.sync.dma_start(
                out=u_fa[64:P], in_=cc_out_a[0:CCU].rearrange(
                    "(p h f) -> p h f", p=64, h=H))
            u_fb = embB.tile([P, H, Dh + 1], bf16)
            nc.sync.dma_start(
                out=u_fb[0:64], in_=cc_out_b[0:CCU].rearrange(
                    "(p h f) -> p h f", p=64, h=H))
            nc.sync.dma_start(
                out=u_fb[64:P], in_=cc_out_b[0:CCU].rearrange(
                    "(p h f) -> p h f", p=64, h=H))
            u_f = embB.tile([P, H, Dh + 1], f32)
            nc.vector.tensor_add(out=u_f, in0=u_fa, in1=u_fb)
            emb_f = embB.tile([1, 2 * D], bf16)
            nc.sync.dma_start(
                out=emb_f, in_=cc_out_a[CCU:CCN].rearrange("(a f) -> a f", a=1))

            rs = embB.tile([P, H, 1], f32)
            nc.vector.reciprocal(out=rs, in_=u_f[:, :, Dh:Dh + 1])
            attn2 = embB.tile([P, KC, P], bf16)
            nc.gpsimd.memset(attn2, 0.0)
            for h in range(H):
                base = 64 * (h % 2)
                nc.vector.tensor_scalar_mul(
                    out=attn2[base:base + 64, h // 2, base:base + 64],
                    in0=u_f[base:base + 64, h, 0:Dh],
                    scalar1=rs[base:base + 64, h, :])

            srow = embB.tile([1, D], f32)
            shrow = embB.tile([1, D], f32)
            if has_embb:
                nc.vector.tensor_add(out=srow, in0=emb_f[:, 0:D],
                                     in1=vec_s[:, 6, :])
                nc.vector.tensor_add(out=shrow, in0=emb_f[:, D:2 * D],
                                     in1=vec_s[:, 7, :])
            else:
                nc.vector.tensor_copy(out=srow, in_=emb_f[:, 0:D])
                nc.vector.tensor_copy(out=shrow, in_=emb_f[:, D:2 * D])
            t1 = embB.tile([1, D], f32)
            nc.vector.tensor_scalar_add(out=t1, in0=srow, scalar1=1.0)
            arow = embB.tile([1, D], f32)
            nc.vector.tensor_mul(out=arow, in0=t1, in1=vec_s[:, 4, :])
            crow = embB.tile([1, D], f32)
            nc.vector.tensor_mul(out=crow, in0=t1, in1=vec_s[:, 5, :])
            nc.vector.tensor_add(out=crow, in0=crow, in1=shrow)

            # transpose a,c rows to per-chunk columns [P, KC]
            acp = psB.tile([P, 2, KC], f32, tag="ac", bufs=1)
            for j in range(KC):
                nc.tensor.transpose(acp[:, 0, j:j + 1],
                                    arow[:, j * P:(j + 1) * P],
                                    one_f32)
                nc.tensor.transpose(acp[:, 1, j:j + 1],
                                    crow[:, j * P:(j + 1) * P],
                                    one_f32)
            a_col = embB.tile([P, KC], f32)
            nc.scalar.copy(out=a_col, in_=acp[:, 0, :])
            c_col = embB.tile([P, KC], f32)
            nc.scalar.copy(out=c_col, in_=acp[:, 1, :])

            # ---- B1: y = q@attn, broadcast 1/S, LN2 stats ----
            for i in range(NT):
                py = psB.tile([P, KC, P], f32, tag="py")
                for j in range(KC):
                    nc.tensor.matmul(out=py[:, j, :], lhsT=q_tiles[i][:, j, :],
                                     rhs=attn2[:, j, :], start=True, stop=True)
                ysb = ystash.tile([P, D], bf16, tag="ysb")
                y_tiles.append(ysb)
                nc.vector.tensor_mul(
                    out=ysb[:].rearrange("p (h d) -> p h d", h=H),
                    in0=py[:].rearrange("p a b -> p (a b)").rearrange(
                        "p (h d) -> p h d", h=H),
                    in1=rq_st[:, i, :, :].to_broadcast([P, H, Dh]))
                # LN2 sums via ACT accumulate (scalar engine is idle here)
                dumm = workB.tile([P, D], bf16, tag="dumm")
                nc.scalar.activation(out=dumm, in_=ysb, func=Act.Identity,
                                     accum_out=s1_st[:, i:i + 1])
                nc.scalar.activation(out=dumm, in_=ysb, func=Act.Square,
                                     accum_out=s2_st[:, i:i + 1])

            # batched LN2 rstd/bias: var = E[y^2] - mu^2
            mu2_a = stat.tile([P, NT], f32)
            nc.vector.tensor_scalar_mul(out=mu2_a, in0=s1_st, scalar1=1.0 / D)
            var2_a = stat.tile([P, NT], f32)
            nc.vector.tensor_scalar_mul(out=var2_a, in0=s2_st, scalar1=1.0 / D)
            musq = stat.tile([P, NT], f32)
            nc.vector.tensor_mul(out=musq, in0=mu2_a, in1=mu2_a)
            nc.vector.tensor_sub(out=var2_a, in0=var2_a, in1=musq)
            nc.scalar.activation(out=sd2_a, in_=var2_a, func=Act.Ln,
                                 bias=eps_t)
            nc.scalar.activation(out=rstd2_a, in_=sd2_a, func=Act.Exp,
                                 scale=-0.5)
            nc.vector.tensor_mul(out=nb2_a, in0=mu2_a, in1=rstd2_a)
            nc.vector.tensor_scalar_mul(out=nb2_a, in0=nb2_a, scalar1=-1.0)

            # ---- B2: normalize, transpose, fused stylize+silu, out proj ----
            for i in range(NT):
                z = workB.tile([P, D], bf16, tag="z")
                nc.vector.tensor_scalar(out=z, in0=y_tiles[i],
                                        scalar1=rstd2_a[:, i:i + 1],
                                        scalar2=nb2_a[:, i:i + 1],
                                        op0=Alu.mult, op1=Alu.add)
                zTp = psB.tile([P, KC, P], bf16, tag="pT")
                for j in range(KC):
                    nc.tensor.transpose(zTp[:, j, :], z[:, j * P:(j + 1) * P],
                                        ident)
                # silu(zT*a + c) per chunk: a,c are per-partition here
                hT = workB.tile([P, KC, P], bf16, tag="hT")
                for j in range(KC):
                    nc.scalar.activation(out=hT[:, j, :], in_=zTp[:, j, :],
                                         func=Act.Silu,
                                         scale=a_col[:, j:j + 1],
                                         bias=c_col[:, j:j + 1])
                po = psB.tile([P, D], f32, tag="po")
                for j in range(KC):
                    nc.tensor.matmul(out=po, lhsT=hT[:, j, :],
                                     rhs=wo_s[:, j, :],
                                     start=(j == 0), stop=(j == KC - 1))
                osb = workB.tile([P, D], f32, tag="osb")
                nc.vector.tensor_add(out=osb, in0=po, in1=x_tiles[i])
                if has_outb:
                    nc.vector.tensor_add(out=osb, in0=osb, in1=ob_bc)
                nc.sync.dma_start(out=y_out[i * P:(i + 1) * P, :], in_=osb)

    nc.compile()
    return nc


def _to_f8(a):
    return np.clip(a * WSCALE, -240.0, 240.0).astype(ml_dtypes.float8_e4m3fn)


def _prep(inputs, flags):
    bf = ml_dtypes.bfloat16
    x = np.asarray(inputs["x"], np.float32)
    emb = np.asarray(inputs["emb"], np.float32)
    src_mask = np.asarray(inputs["src_mask"], np.float32)
    gamma = np.asarray(inputs["gamma"], np.float32)
    beta = np.asarray(inputs["beta"], np.float32)
    gamma2 = np.asarray(inputs["gamma2"], np.float32)
    beta2 = np.asarray(inputs["beta2"], np.float32)
    emb_b = np.asarray(inputs["emb_b"], np.float32)
    out_b = np.asarray(inputs["out_b"], np.float32)

    def foldW(Wname):
        W = np.asarray(inputs[Wname], np.float32)
        return np.ascontiguousarray(_to_f8(gamma[:, None] * W).reshape(KC, P, D))

    wq, wk, wv = foldW("Wq"), foldW("Wk"), foldW("Wv")
    wo = np.ascontiguousarray(
        np.asarray(inputs["out_W"], np.float32).astype(bf).reshape(KC, P, D))
    bq_f = np.asarray(inputs["bq"], np.float32) + beta @ np.asarray(inputs["Wq"], np.float32)
    bk_f = np.asarray(inputs["bk"], np.float32) + beta @ np.asarray(inputs["Wk"], np.float32)
    bv_f = np.asarray(inputs["bv"], np.float32) + beta @ np.asarray(inputs["Wv"], np.float32)
    vecs = np.ascontiguousarray(np.stack(
        [bq_f, bk_f, bv_f, out_b, gamma2, beta2, emb_b[:D], emb_b[D:]]
    ).astype(np.float32).reshape(1, 8, D))
    emb_W = np.asarray(inputs["emb_W"], np.float32)
    we_halves = [
        np.ascontiguousarray(
            emb_W[t * TEH:(t + 1) * TEH].astype(bf).reshape(TEC, P, 2 * D))
        for t in range(2)]

    in_maps = []
    for c in range(NCORES):
        b, th = c // 2, c % 2
        sl = slice(th * TH, (th + 1) * TH)
        in_maps.append({
            "x": np.ascontiguousarray(x[b, sl]),
            "mask": np.ascontiguousarray(src_mask[b, sl, 0]),
            "embv": np.ascontiguousarray(emb[b, th * TEH:(th + 1) * TEH]),
            "wq": wq, "wk": wk, "wv": wv, "wo": wo,
            "we": we_halves[th],
            "vecs": vecs,
        })
    return in_maps


def _flags(inputs):
    beta = np.asarray(inputs["beta"], np.float32)

    def nz(v):
        return bool(np.any(np.asarray(v) != 0))

    bq_f = np.asarray(inputs["bq"], np.float32) + beta @ np.asarray(inputs["Wq"], np.float32)
    bk_f = np.asarray(inputs["bk"], np.float32) + beta @ np.asarray(inputs["Wk"], np.float32)
    bv_f = np.asarray(inputs["bv"], np.float32) + beta @ np.asarray(inputs["Wv"], np.float32)
    return (nz(bq_f), nz(bk_f), nz(bv_f), nz(inputs["out_b"]), nz(inputs["emb_b"]))


def get_nc_and_inmaps(**inputs):
    flags = _flags(inputs)
    if flags not in _CACHE:
        _CACHE[flags] = _build(flags)
    return _CACHE[flags], _prep(inputs, flags)


def kernel(**inputs):
    from concourse.bass_utils import run_bass_kernel_spmd
    nc, in_maps = get_nc_and_inmaps(**inputs)
    res = run_bass_kernel_spmd(nc, in_maps, list(range(NCORES)))
    out = np.empty((B, T, D), np.float32)
    for c in range(NCORES):
        b, th = c // 2, c % 2
        out[b, th * TH:(th + 1) * TH] = res.results[c]["y"]
    return out


# revision 25
# speedup vs baseline: 2.7481x; 1.0825x over previous
"""Trainium2 Bass kernel for nn_LinearTemporalSelfAttention (B=4,T=8192,D=512,H=8).

Sharding: 8 cores = B(4) x T-halves(2). Each core owns a (b, t-half) slab
(4096 x 512) end-to-end. Cross-core data is only the KV-state einsum
(sum over full T) and the emb projection (emb_W sharded over TE within a
pair) — folded into two pair-wise bf16 AllReduces (first half of U mid-
phase-A so it overlaps compute, second half at the end).

v3 structure:
 - LN rstd batched per 16-tile group (one Ln + one Exp each) so the
   scalar engine never thrashes activation tables; groups interleave
   with the projection loop so stats DMA/vector work overlaps PE work.
 - All transposes on the PE (matmul is_transpose) + PSUM evac.
 - QKV / out projections and the KV-state einsum run fp8(e4m3)
   DoubleRow (2 k-subtiles per pass). Weights x64 on host; exp(k),
   masked v quantized to fp8 (their errors average out over T in U).
 - Per-head softmax-q 1/S applied in ONE broadcast tensor_tensor op.
 - Phase B pass 2 works in transposed space: PE-transpose LN2-normalized
   y, then a single ACT Silu per chunk applies the stylization scale
   and shift (per-partition there) while evacuating PSUM into fp8.
 - Elementwise work spread across scalar/vector/gpsimd.
"""
import numpy as np
import ml_dtypes

B, T, D, H, TE = 4, 8192, 512, 8, 2048
Dh = D // H          # 64
EPS = 1e-5
NCORES = 8
TH = T // 2          # 4096 rows per core
P = 128
NT = TH // P         # 32 row tiles
NG = 16              # tiles per stat/U group
KC = D // P          # 4 contraction chunks
TEH = TE // 2        # 1024 te rows per core
TEC = TEH // P       # 8 te chunks
CCU = 64 * H * (Dh + 1)     # 33280 floats of U_aug
CCN = CCU + 2 * D           # + emb partial
WSCALE = 64.0        # fp8 weight prescale
RWS = 1.0 / WSCALE

_CACHE: dict = {}


def _build(flags):
    has_bq, has_bk, has_bv, has_outb, has_embb = flags
    from contextlib import ExitStack
    import concourse.bass as bass
    import concourse.bacc as bacc
    import concourse.tile as tile
    import concourse.mybir as mybir
    from concourse.masks import make_identity

    f32 = mybir.dt.float32
    bf16 = mybir.dt.bfloat16
    f8 = mybir.dt.float8e4
    Alu = mybir.AluOpType
    Act = mybir.ActivationFunctionType
    DR = mybir.MatmulPerfMode.DoubleRow

    nc = bacc.Bacc("TRN2", target_bir_lowering=False, debug=False,
                   enable_asserts=True, num_devices=NCORES)

    x_in = nc.declare_dram_parameter("x", [TH, D], f32, isOutput=False)
    mk_in = nc.declare_dram_parameter("mask", [TH], f32, isOutput=False)
    emb_in = nc.declare_dram_parameter("embv", [TEH], f32, isOutput=False)
    wq_in = nc.declare_dram_parameter("wq", [KC, P, D], f8, isOutput=False)
    wk_in = nc.declare_dram_parameter("wk", [KC, P, D], f8, isOutput=False)
    wv_in = nc.declare_dram_parameter("wv", [KC, P, D], f8, isOutput=False)
    wo_in = nc.declare_dram_parameter("wo", [KC, P, D], bf16, isOutput=False)
    we_in = nc.declare_dram_parameter("we", [TEC, P, 2 * D], bf16, isOutput=False)
    vec_in = nc.declare_dram_parameter("vecs", [1, 8, D], f32, isOutput=False)
    y_out = nc.declare_dram_parameter("y", [TH, D], f32, isOutput=True)

    PAIRS = [[0, 1], [2, 3], [4, 5], [6, 7]]

    with tile.TileContext(nc) as tc, ExitStack() as ctx:
        const = ctx.enter_context(tc.tile_pool(name="const", bufs=1))
        wpool = ctx.enter_context(tc.tile_pool(name="wpool", bufs=1))
        xstash = ctx.enter_context(tc.tile_pool(name="xstash", bufs=NT))
        qstash = ctx.enter_context(tc.tile_pool(name="qstash", bufs=NT))
        ystash = ctx.enter_context(tc.tile_pool(name="ystash", bufs=NT))
        stat = ctx.enter_context(tc.tile_pool(name="stat", bufs=1))
        dramp = ctx.enter_context(tc.tile_pool(name="dram", bufs=1, space="DRAM"))

        ident = const.tile([P, P], bf16)
        make_identity(nc, ident)
        eps_t = const.tile([P, 1], f32)
        nc.vector.memset(eps_t, EPS)
        ones8 = const.tile([P, H, 1], bf16)
        nc.vector.memset(ones8, 1.0)
        ones_row = const.tile([1, P], bf16)
        nc.vector.memset(ones_row, 1.0)
        one_f32 = const.tile([1, 1], f32)
        nc.vector.memset(one_f32, 1.0)

        wq_s = wpool.tile([P, KC, D], f8)
        nc.sync.dma_start(out=wq_s, in_=wq_in[:].rearrange("c p d -> p c d"))
        wk_s = wpool.tile([P, KC, D], f8)
        nc.sync.dma_start(out=wk_s, in_=wk_in[:].rearrange("c p d -> p c d"))
        wv_s = wpool.tile([P, KC, D], f8)
        nc.sync.dma_start(out=wv_s, in_=wv_in[:].rearrange("c p d -> p c d"))
        wo_s = wpool.tile([P, KC, D], bf16)
        nc.sync.dma_start(out=wo_s, in_=wo_in[:].rearrange("c p d -> p c d"))
        we_s = wpool.tile([P, TEC, 2 * D], bf16)
        nc.sync.dma_start(out=we_s, in_=we_in[:].rearrange("c p d -> p c d"))
        mask_s = wpool.tile([P, NT], f32)
        nc.sync.dma_start(out=mask_s, in_=mk_in[:].rearrange("(n p) -> p n", p=P))
        vec_s = wpool.tile([1, 8, D], f32)
        nc.sync.dma_start(out=vec_s, in_=vec_in[:])

        mv_st = stat.tile([P, NT, 2], f32)
        s1_st = stat.tile([P, NT], f32)
        s2_st = stat.tile([P, NT], f32)
        rq_st = stat.tile([P, NT, H, 1], f32)
        rstd_a = stat.tile([P, NT], f32)
        nb_a = stat.tile([P, NT], f32)
        sd_a = stat.tile([P, NT], f32)
        rstd2_a = stat.tile([P, NT], f32)
        nb2_a = stat.tile([P, NT], f32)
        sd2_a = stat.tile([P, NT], f32)

        cc_in_a = dramp.tile([CCN], bf16)
        cc_out_a = dramp.tile([CCN], bf16)
        cc_in_b = dramp.tile([CCU], bf16)
        cc_out_b = dramp.tile([CCU], bf16)

        x_tiles = []
        q_tiles = []
        y_tiles = []

        with ExitStack() as ctxA:
            work = ctxA.enter_context(tc.tile_pool(name="work", bufs=3))
            psP = ctxA.enter_context(tc.tile_pool(name="psP", bufs=1, space="PSUM"))
            psT = ctxA.enter_context(tc.tile_pool(name="psT", bufs=2, space="PSUM"))
            psU = ctxA.enter_context(tc.tile_pool(name="psU", bufs=1, space="PSUM"))
            embp = ctxA.enter_context(tc.tile_pool(name="embp", bufs=1))

            # ---- bias broadcast tiles (only when biases nonzero) ----
            def bcast_row(row_idx, name):
                pb = psT.tile([P, KC, P], f32, tag="pT")
                rbf = const.tile([1, D], bf16, tag="rbf_" + name)
                nc.vector.tensor_copy(out=rbf, in_=vec_s[:, row_idx, :])
                nc.tensor.matmul(out=pb[:].rearrange("p a b -> p (a b)"),
                                 lhsT=ones_row, rhs=rbf, start=True, stop=True)
                bc = const.tile([P, D], f32, tag="bc_" + name)
                nc.scalar.copy(out=bc, in_=pb[:].rearrange("p a b -> p (a b)"))
                return bc

            bq_bc = bcast_row(0, "bq") if has_bq else None
            bk_bc = bcast_row(1, "bk") if has_bk else None
            bv_bc = bcast_row(2, "bv") if has_bv else None
            ob_bc = bcast_row(3, "ob") if has_outb else None

            # ---- emb projection partial (this core's TE shard) ----
            # silu via exp-table ops only: e*sigmoid(e) = e/(1+exp(-e))
            embt = embp.tile([P, TEC], f32)
            nc.sync.dma_start(out=embt, in_=emb_in[:].rearrange("(c p) -> p c", p=P))
            emneg = embp.tile([P, TEC], f32)
            nc.scalar.activation(out=emneg, in_=embt, func=Act.Exp, scale=-1.0)
            nc.vector.tensor_scalar_add(out=emneg, in0=emneg, scalar1=1.0)
            nc.vector.reciprocal(out=emneg, in_=emneg)
            embs = embp.tile([P, TEC], bf16)
            nc.vector.tensor_mul(out=embs, in0=embt, in1=emneg)
            pe0 = psP.tile([P, D], f32, tag="pq")
            pe1 = psP.tile([P, D], f32, tag="pk")
            for j in range(TEC):
                nc.tensor.matmul(out=pe0[0:1, :], lhsT=embs[:, j:j + 1],
                                 rhs=we_s[:, j, 0:D],
                                 start=(j == 0), stop=(j == TEC - 1))
            for j in range(TEC):
                nc.tensor.matmul(out=pe1[0:1, :], lhsT=embs[:, j:j + 1],
                                 rhs=we_s[:, j, D:2 * D],
                                 start=(j == 0), stop=(j == TEC - 1))
            emb_part = embp.tile([1, 2 * D], bf16)
            nc.scalar.copy(out=emb_part[:, 0:D], in_=pe0[0:1, :])
            nc.scalar.copy(out=emb_part[:, D:2 * D], in_=pe1[0:1, :])

            u0 = psU.tile([P, 2, 2 * (Dh + 1)], f32, tag="u0")
            u1 = psU.tile([P, 2, 2 * (Dh + 1)], f32, tag="u1")
            usb = [embp.tile([64, H, Dh + 1], bf16, tag=f"usb{g}",
                             name=f"usb{g}")
                   for g in range(2)]

            def stats_tile(i):
                xt = xstash.tile([P, D], f32, tag="x")
                x_tiles.append(xt)
                nc.sync.dma_start(out=xt, in_=x_in[i * P:(i + 1) * P, :])
                st = work.tile([P, 6], f32, tag="st")
                nc.vector.bn_stats(out=st, in_=xt)
                nc.vector.bn_aggr(out=mv_st[:, i, :], in_=st)

            def stats_batch(g):
                sl = slice(g * 8, (g + 1) * 8)
                nc.scalar.activation(out=sd_a[:, sl], in_=mv_st[:, sl, 1],
                                     func=Act.Ln, bias=eps_t)
                nc.scalar.activation(out=rstd_a[:, sl], in_=sd_a[:, sl],
                                     func=Act.Exp, scale=-0.5)
                nc.vector.tensor_mul(out=nb_a[:, sl], in0=mv_st[:, sl, 0],
                                     in1=rstd_a[:, sl])
                nc.vector.tensor_scalar_mul(out=nb_a[:, sl], in0=nb_a[:, sl],
                                            scalar1=-1.0)

            pair_state = {}

            def proj_tile(i):
                # normalize on gpsimd (scalar tables untouched)
                xn = work.tile([P, D], bf16, tag="xn")
                nc.gpsimd.tensor_scalar(out=xn, in0=x_tiles[i],
                                        scalar1=rstd_a[:, i:i + 1],
                                        scalar2=nb_a[:, i:i + 1],
                                        op0=Alu.mult, op1=Alu.add)
                xTp = psT.tile([P, KC, P], bf16, tag="pT")
                for j in range(KC):
                    nc.tensor.transpose(xTp[:, j, :], xn[:, j * P:(j + 1) * P],
                                        ident)
                xT = work.tile([P, KC, P], f8, tag="xT")
                for j in range(KC):
                    if j % 2 == 0:
                        nc.scalar.copy(out=xT[:, j, :], in_=xTp[:, j, :])
                    else:
                        nc.vector.tensor_copy(out=xT[:, j, :], in_=xTp[:, j, :])

                pq = psP.tile([P, D], f32, tag="pq")
                pk = psP.tile([P, D], f32, tag="pk")
                pv = psP.tile([P, D], f32, tag="pv")
                for m in range(2):
                    nc.tensor.matmul(out=pq, lhsT=xT[:, 2 * m:2 * m + 2, :],
                                     rhs=wq_s[:, 2 * m:2 * m + 2, :],
                                     start=(m == 0), stop=(m == 1),
                                     perf_mode=DR)
                for m in range(2):
                    nc.tensor.matmul(out=pk, lhsT=xT[:, 2 * m:2 * m + 2, :],
                                     rhs=wk_s[:, 2 * m:2 * m + 2, :],
                                     start=(m == 0), stop=(m == 1),
                                     perf_mode=DR)
                for m in range(2):
                    nc.tensor.matmul(out=pv, lhsT=xT[:, 2 * m:2 * m + 2, :],
                                     rhs=wv_s[:, 2 * m:2 * m + 2, :],
                                     start=(m == 0), stop=(m == 1),
                                     perf_mode=DR)
                if has_bq:
                    nc.vector.scalar_tensor_tensor(
                        out=pq, in0=pq, scalar=RWS, in1=bq_bc,
                        op0=Alu.mult, op1=Alu.add)
                if has_bk:
                    nc.vector.scalar_tensor_tensor(
                        out=pk, in0=pk, scalar=RWS, in1=bk_bc,
                        op0=Alu.mult, op1=Alu.add)
                if has_bv:
                    nc.vector.scalar_tensor_tensor(
                        out=pv, in0=pv, scalar=RWS, in1=bv_bc,
                        op0=Alu.mult, op1=Alu.add)
                qsc = 1.0 if has_bq else RWS
                ksc = 1.0 if has_bk else RWS
                vsc = 1.0 if has_bv else RWS

                # q: exp, per-head sums, PE transpose into qT stash
                qt = work.tile([P, D], bf16, tag="qt")
                nc.scalar.activation(out=qt, in_=pq, func=Act.Exp, scale=qsc)
                qs = work.tile([P, H, 1], f32, tag="qs")
                nc.vector.reduce_sum(
                    out=qs, in_=qt[:].rearrange("p (h d) -> p h d", h=H),
                    axis=mybir.AxisListType.X)
                nc.vector.reciprocal(out=rq_st[:, i, :, :], in_=qs)
                qTp = psT.tile([P, KC, P], bf16, tag="pT")
                for j in range(KC):
                    nc.tensor.transpose(qTp[:, j, :], qt[:, j * P:(j + 1) * P],
                                        ident)
                qT = qstash.tile([P, KC, P], bf16, tag="qT")
                q_tiles.append(qT)
                for j in range(KC):
                    if j % 2 == 0:
                        nc.scalar.copy(out=qT[:, j, :], in_=qTp[:, j, :])
                    else:
                        nc.vector.tensor_copy(out=qT[:, j, :], in_=qTp[:, j, :])

                # k, v bf16; U einsum per head-pair
                et = work.tile([P, D], bf16, tag="et")
                nc.scalar.activation(out=et, in_=pk, func=Act.Exp, scale=ksc)
                va = work.tile([P, H, Dh + 1], bf16, tag="va")
                nc.vector.tensor_scalar(
                    out=va[:, :, 0:Dh],
                    in0=pv[:].rearrange("p (h d) -> p h d", h=H),
                    scalar1=mask_s[:, i:i + 1], scalar2=vsc,
                    op0=Alu.mult, op1=Alu.mult)
                nc.gpsimd.tensor_scalar_mul(out=va[:, :, Dh:Dh + 1],
                                            in0=ones8,
                                            scalar1=mask_s[:, i:i + 1])
                ig = i % NG
                for p2 in range(4):
                    u = u0 if p2 < 2 else u1
                    nc.tensor.matmul(
                        out=u[:, p2 % 2, :],
                        lhsT=et[:, p2 * P:(p2 + 1) * P],
                        rhs=va[:, 2 * p2:2 * p2 + 2, :].rearrange(
                            "p h f -> p (h f)"),
                        start=(ig == 0 and p2 % 2 == 0),
                        stop=(ig == NG - 1 and p2 % 2 == 1))

            def ship_u(g):
                u_sb = usb[g]
                for p2 in range(4):
                    u = u0 if p2 < 2 else u1
                    nc.scalar.copy(out=u_sb[:, 2 * p2, :],
                                   in_=u[0:64, p2 % 2, 0:Dh + 1])
                    nc.scalar.copy(out=u_sb[:, 2 * p2 + 1, :],
                                   in_=u[64:P, p2 % 2, Dh + 1:2 * Dh + 2])
                cc_in = cc_in_a if g == 0 else cc_in_b
                cc_out = cc_out_a if g == 0 else cc_out_b
                nc.sync.dma_start(
                    out=cc_in[0:CCU].rearrange("(p h f) -> p h f", p=64, h=H),
                    in_=u_sb)
                if g == 0:
                    nc.sync.dma_start(
                        out=cc_in[CCU:CCN].rearrange("(a f) -> a f", a=1),
                        in_=emb_part)
                nc.gpsimd.collective_compute(
                    "AllReduce", Alu.add, replica_groups=PAIRS,
                    ins=[cc_in[:]], outs=[cc_out[:]])

            # 8-tile stat groups; stats of group g+1 interleave with
            # projections of group g. U halves AllReduce at tiles 15/31.
            for i in range(8):
                stats_tile(i)
            stats_batch(0)
            for g in range(3):
                for k in range(8):
                    stats_tile(8 * (g + 1) + k)
                    proj_tile(8 * g + k)
                stats_batch(g + 1)
                if g == 1:
                    ship_u(0)
            for i in range(24, NT):
                proj_tile(i)
            ship_u(1)

        # ---- phase B prologue: attn state + stylization vectors ----
        with ExitStack() as ctxB:
            workB = ctxB.enter_context(tc.tile_pool(name="workB", bufs=3))
            psB = ctxB.enter_context(tc.tile_pool(name="psB", bufs=2, space="PSUM"))
            embB = ctxB.enter_context(tc.tile_pool(name="embB", bufs=1))

            u_fa = embB.tile([P, H, Dh + 1], bf16)
            nc.sync.dma_start(
                out=u_fa[0:64], in_=cc_out_a[0:CCU].rearrange(
                    "(p h f) -> p h f", p=64, h=H))
            nc.sync.dma_start(
                out=u_fa[64:P], in_=cc_out_a[0:CCU].rearrange(
                    "(p h f) -> p h f", p=64, h=H))
            u_fb = embB.tile([P, H, Dh + 1], bf16)
            nc.sync.dma_start(
                out=u_fb[0:64], in_=cc_out_b[0:CCU].rearrange(
                    "(p h f) -> p h f", p=64, h=H))
            nc.sync.dma_start(
                out=u_fb[64:P], in_=cc_out_b[0:CCU].rearrange(
                    "(p h f) -> p h f", p=64, h=H))
            u_f = embB.tile([P, H, Dh + 1], f32)
            nc.vector.tensor_add(out=u_f, in0=u_fa, in1=u_fb)
            emb_f = embB.tile([1, 2 * D], bf16)
            nc.sync.dma_start(
                out=emb_f, in_=cc_out_a[CCU:CCN].rearrange("(a f) -> a f", a=1))

            rs = embB.tile([P, H, 1], f32)
            nc.vector.reciprocal(out=rs, in_=u_f[:, :, Dh:Dh + 1])
            attn2 = embB.tile([P, KC, P], bf16)
            nc.gpsimd.memset(attn2, 0.0)
            for h in range(H):
                base = 64 * (h % 2)
                nc.vector.tensor_scalar_mul(
                    out=attn2[base:base + 64, h // 2, base:base + 64],
                    in0=u_f[base:base + 64, h, 0:Dh],
                    scalar1=rs[base:base + 64, h, :])

            srow = embB.tile([1, D], f32)
            shrow = embB.tile([1, D], f32)
            if has_embb:
                nc.vector.tensor_add(out=srow, in0=emb_f[:, 0:D],
                                     in1=vec_s[:, 6, :])
                nc.vector.tensor_add(out=shrow, in0=emb_f[:, D:2 * D],
                                     in1=vec_s[:, 7, :])
            else:
                nc.vector.tensor_copy(out=srow, in_=emb_f[:, 0:D])
                nc.vector.tensor_copy(out=shrow, in_=emb_f[:, D:2 * D])
            t1 = embB.tile([1, D], f32)
            nc.vector.tensor_scalar_add(out=t1, in0=srow, scalar1=1.0)
            arow = embB.tile([1, D], f32)
            nc.vector.tensor_mul(out=arow, in0=t1, in1=vec_s[:, 4, :])
            crow = embB.tile([1, D], f32)
            nc.vector.tensor_mul(out=crow, in0=t1, in1=vec_s[:, 5, :])
            nc.vector.tensor_add(out=crow, in0=crow, in1=shrow)

            # transpose a,c rows to per-chunk columns [P, KC]
            acp = psB.tile([P, 2, KC], f32, tag="ac", bufs=1)
            for j in range(KC):
                nc.tensor.transpose(acp[:, 0, j:j + 1],
                                    arow[:, j * P:(j + 1) * P],
                                    one_f32)
                nc.tensor.transpose(acp[:, 1, j:j + 1],
                                    crow[:, j * P:(j + 1) * P],
                                    one_f32)
            a_col = embB.tile([P, KC], f32)
            nc.scalar.copy(out=a_col, in_=acp[:, 0, :])
            c_col = embB.tile([P, KC], f32)
            nc.scalar.copy(out=c_col, in_=acp[:, 1, :])

            # ---- B1: y = q@attn, broadcast 1/S, LN2 stats ----
            for i in range(NT):
                py = psB.tile([P, KC, P], f32, tag="py")
                for j in range(KC):
                    nc.tensor.matmul(out=py[:, j, :], lhsT=q_tiles[i][:, j, :],
                                     rhs=attn2[:, j, :], start=True, stop=True)
                ysb = ystash.tile([P, D], bf16, tag="ysb")
                y_tiles.append(ysb)
                # evac with broadcast 1/S; accum gives sum(y) for LN2 free
                nc.vector.scalar_tensor_tensor(
                    out=ysb[:].rearrange("p (h d) -> p h d", h=H),
                    in0=py[:].rearrange("p a b -> p (a b)").rearrange(
                        "p (h d) -> p h d", h=H),
                    scalar=1.0,
                    in1=rq_st[:, i, :, :].to_broadcast([P, H, Dh]),
                    op0=Alu.mult, op1=Alu.mult,
                    accum_out=s1_st[:, i:i + 1])
                # sum(y^2) via ACT Square accumulate (scalar is idle here)
                dumm = workB.tile([P, D], bf16, tag="dumm")
                nc.scalar.activation(out=dumm, in_=ysb, func=Act.Square,
                                     accum_out=s2_st[:, i:i + 1])

            # batched LN2 rstd/bias: var = E[y^2] - mu^2
            mu2_a = stat.tile([P, NT], f32)
            nc.vector.tensor_scalar_mul(out=mu2_a, in0=s1_st, scalar1=1.0 / D)
            var2_a = stat.tile([P, NT], f32)
            nc.vector.tensor_scalar_mul(out=var2_a, in0=s2_st, scalar1=1.0 / D)
            musq = stat.tile([P, NT], f32)
            nc.vector.tensor_mul(out=musq, in0=mu2_a, in1=mu2_a)
            nc.vector.tensor_sub(out=var2_a, in0=var2_a, in1=musq)
            nc.scalar.activation(out=sd2_a, in_=var2_a, func=Act.Ln,
                                 bias=eps_t)
            nc.scalar.activation(out=rstd2_a, in_=sd2_a, func=Act.Exp,
                                 scale=-0.5)
            nc.vector.tensor_mul(out=nb2_a, in0=mu2_a, in1=rstd2_a)
            nc.vector.tensor_scalar_mul(out=nb2_a, in0=nb2_a, scalar1=-1.0)

            # ---- B2: normalize, transpose, fused stylize+silu, out proj ----
            for i in range(NT):
                z = workB.tile([P, D], bf16, tag="z")
                nc.vector.tensor_scalar(out=z, in0=y_tiles[i],
                                        scalar1=rstd2_a[:, i:i + 1],
                                        scalar2=nb2_a[:, i:i + 1],
                                        op0=Alu.mult, op1=Alu.add)
                zTp = psB.tile([P, KC, P], bf16, tag="pT")
                for j in range(KC):
                    nc.tensor.transpose(zTp[:, j, :], z[:, j * P:(j + 1) * P],
                                        ident)
                # silu(zT*a + c) per chunk: a,c are per-partition here
                hT = workB.tile([P, KC, P], bf16, tag="hT")
                for j in range(KC):
                    nc.scalar.activation(out=hT[:, j, :], in_=zTp[:, j, :],
                                         func=Act.Silu,
                                         scale=a_col[:, j:j + 1],
                                         bias=c_col[:, j:j + 1])
                po = psB.tile([P, D], f32, tag="po")
                for j in range(KC):
                    nc.tensor.matmul(out=po, lhsT=hT[:, j, :],
                                     rhs=wo_s[:, j, :],
                                     start=(j == 0), stop=(j == KC - 1))
                osb = workB.tile([P, D], f32, tag="osb")
                nc.vector.tensor_add(out=osb, in0=po, in1=x_tiles[i])
                if has_outb:
                    nc.vector.tensor_add(out=osb, in0=osb, in1=ob_bc)
                nc.sync.dma_start(out=y_out[i * P:(i + 1) * P, :], in_=osb)

    nc.compile()
    return nc


def _to_f8(a):
    return np.clip(a * WSCALE, -240.0, 240.0).astype(ml_dtypes.float8_e4m3fn)


def _prep(inputs, flags):
    bf = ml_dtypes.bfloat16
    x = np.asarray(inputs["x"], np.float32)
    emb = np.asarray(inputs["emb"], np.float32)
    src_mask = np.asarray(inputs["src_mask"], np.float32)
    gamma = np.asarray(inputs["gamma"], np.float32)
    beta = np.asarray(inputs["beta"], np.float32)
    gamma2 = np.asarray(inputs["gamma2"], np.float32)
    beta2 = np.asarray(inputs["beta2"], np.float32)
    emb_b = np.asarray(inputs["emb_b"], np.float32)
    out_b = np.asarray(inputs["out_b"], np.float32)

    def foldW(Wname):
        W = np.asarray(inputs[Wname], np.float32)
        return np.ascontiguousarray(_to_f8(gamma[:, None] * W).reshape(KC, P, D))

    wq, wk, wv = foldW("Wq"), foldW("Wk"), foldW("Wv")
    wo = np.ascontiguousarray(
        np.asarray(inputs["out_W"], np.float32).astype(bf).reshape(KC, P, D))
    bq_f = np.asarray(inputs["bq"], np.float32) + beta @ np.asarray(inputs["Wq"], np.float32)
    bk_f = np.asarray(inputs["bk"], np.float32) + beta @ np.asarray(inputs["Wk"], np.float32)
    bv_f = np.asarray(inputs["bv"], np.float32) + beta @ np.asarray(inputs["Wv"], np.float32)
    vecs = np.ascontiguousarray(np.stack(
        [bq_f, bk_f, bv_f, out_b, gamma2, beta2, emb_b[:D], emb_b[D:]]
    ).astype(np.float32).reshape(1, 8, D))
    emb_W = np.asarray(inputs["emb_W"], np.float32)
    we_halves = [
        np.ascontiguousarray(
            emb_W[t * TEH:(t + 1) * TEH].astype(bf).reshape(TEC, P, 2 * D))
        for t in range(2)]

    in_maps = []
    for c in range(NCORES):
        b, th = c // 2, c % 2
        sl = slice(th * TH, (th + 1) * TH)
        in_maps.append({
            "x": np.ascontiguousarray(x[b, sl]),
            "mask": np.ascontiguousarray(src_mask[b, sl, 0]),
            "embv": np.ascontiguousarray(emb[b, th * TEH:(th + 1) * TEH]),
            "wq": wq, "wk": wk, "wv": wv, "wo": wo,
            "we": we_halves[th],
            "vecs": vecs,
        })
    return in_maps


def _flags(inputs):
    beta = np.asarray(inputs["beta"], np.float32)

    def nz(v):
        return bool(np.any(np.asarray(v) != 0))

    bq_f = np.asarray(inputs["bq"], np.float32) + beta @ np.asarray(inputs["Wq"], np.float32)
    bk_f = np.asarray(inputs["bk"], np.float32) + beta @ np.asarray(inputs["Wk"], np.float32)
    bv_f = np.asarray(inputs["bv"], np.float32) + beta @ np.asarray(inputs["Wv"], np.float32)
    return (nz(bq_f), nz(bk_f), nz(bv_f), nz(inputs["out_b"]), nz(inputs["emb_b"]))


def get_nc_and_inmaps(**inputs):
    flags = _flags(inputs)
    if flags not in _CACHE:
        _CACHE[flags] = _build(flags)
    return _CACHE[flags], _prep(inputs, flags)


def kernel(**inputs):
    from concourse.bass_utils import run_bass_kernel_spmd
    nc, in_maps = get_nc_and_inmaps(**inputs)
    res = run_bass_kernel_spmd(nc, in_maps, list(range(NCORES)))
    out = np.empty((B, T, D), np.float32)
    for c in range(NCORES):
        b, th = c // 2, c % 2
        out[b, th * TH:(th + 1) * TH] = res.results[c]["y"]
    return out
